# revision 33
# baseline (speedup 1.0000x reference)
"""Trainium2 Bass kernel for a 3-layer GCN (DeepGRL) on 8 NeuronCores.

Sharding: nodes are sharded contiguously across the 8 cores; edges are owned
by the core that owns their destination node; weights/BN params replicated.

The shipped kernel (build_kernel2) uses aggregate-first ordering,
out = (A_hat @ z) @ W, which needs one less table AllGather than the
multiply-first form and no layer-1 collective at all:

  - Every core holds a full node-major DRAM table of y = dinv * z rows
    (bf16).  Layer 1's table is just dinv * x, staged replicated as an
    ExternalInput - no collective.  Layer 3 produces only local output
    rows - no collective.  Only the two BN layers AllGather their tables
    (in two halves so gathers on half A overlap the AllGather of half B).
  - Aggregation per 128-dst-node block: edge source rows are fetched with
    the SWDGE dma_gather instruction (128 rows per chunk land on 128
    partitions); a one-hot segment matrix S (vector-engine is_equal of
    dst-local ids vs an iota row) is the matmul rhs, so the PSUM
    accumulation emits aggT feature-major.  The self-loop term enters the
    same accumulation as one identity matmul of the local y block.
  - aggT feeds the dense matmul directly as lhsT (no transposes anywhere):
    u = aggT^T @ W_eff -> node-major, scaled per-partition by dinv_dst.
  - BatchNorm (training-mode, biased var) batch stats are ones-vector
    matmuls, AllReduce'd across cores.  Using relu(A*u+B) = A*relu(u+C),
    C = B/A (A = g*rsqrt(var+eps) > 0), the per-feature scale A is folded
    exactly into the next layer's weight rows (f32 activation scale) and
    only the bias C is applied node-major (one DVE add against a [128,128]
    broadcast tile built with two small PE matmuls), then ReLU + dinv
    pre-scale in one activation.

dma_gather indices are int16, so the gathered table is addressed in two
halves (rows < HALF and rows >= HALF); every dst block's edge list is split
into a "lo" and a "hi" sublist, each padded to a multiple of 128.

Measured (marginal repeat=8 vs repeat=1 device time per network pass):
1237 us for the previous multiply-first kernel -> ~930-990 us for this one;
collectives are ~70 us of that, gather+segment-matmul pipeline ~790 us.
The gather pipeline is PER-DESCRIPTOR bound, not byte bound: a timing-only
variant moving the same bytes with half the descriptors (512B each) ran
~320 us/pass faster (~2.4 ns marginal cost per 256B descriptor).  Random
edges need one descriptor per edge, so this is the floor for this layout;
edge lists are source-sorted (sort_src=True) for slightly better HBM
locality.  dma_gather(single_packet=True) hangs the device - never use it.
"""

import math
from contextlib import ExitStack

import numpy as np

import concourse.bacc as bacc
import concourse.bass as bass
import concourse.mybir as mybir
import concourse.tile as tile
from concourse import library_config
from concourse.bass_utils import run_bass_kernel_spmd

P = 128
F32 = mybir.dt.float32
BF16 = mybir.dt.bfloat16
I16 = mybir.dt.int16
AF = mybir.ActivationFunctionType
ALU = mybir.AluOpType


# ----------------------------------------------------------------------------
# Host-side graph preprocessing
# ----------------------------------------------------------------------------
def make_plan(edge_index, N, n_cores=8, nblkA=25, gb=4, sort_src=False):
    """Partition edges by destination core, build per-core gather index /
    segment-id arrays (compile-time constants of the kernel).

    The u table is split into two AllGather'd halves by per-core block
    range: blocks [0, nblkA) -> table A, blocks [nblkA, nblk) -> table B.
    Gathers from table A can start as soon as AllGather-A lands, while
    AllGather-B is still in flight.  (lo == A, hi == B throughout.)"""
    src = np.asarray(edge_index[0], dtype=np.int64)
    dst = np.asarray(edge_index[1], dtype=np.int64)
    E = src.shape[0]

    indeg = np.bincount(dst, minlength=N).astype(np.float64)
    deg = indeg + 1.0  # self loop
    dinv = (1.0 / np.sqrt(deg)).astype(np.float32)

    npc = N // n_cores
    assert npc * n_cores == N
    nblk = math.ceil(npc / P)
    npc_pad = nblk * P
    nblkB = nblk - nblkA
    rA = nblkA * P  # per-core rows in table A
    rB = nblkB * P
    assert n_cores * rA < 32768 and n_cores * rB < 32768  # int16 gather idx

    dst_core = dst // npc
    dloc = dst - dst_core * npc

    # source row in the two-table layout
    src_core = src // npc
    src_loc = src - src_core * npc

    # bucket edges: per (core, block) -> lo (table A) / hi (table B) list
    lo_lists = [[[] for _ in range(nblk)] for _ in range(n_cores)]
    hi_lists = [[[] for _ in range(nblk)] for _ in range(n_cores)]
    lo_dl = [[[] for _ in range(nblk)] for _ in range(n_cores)]
    hi_dl = [[[] for _ in range(nblk)] for _ in range(n_cores)]
    order = np.argsort(dst, kind="stable")
    for e in order:
        r = dst_core[e]
        b = dloc[e] // P
        d_in_blk = dloc[e] - b * P
        sl = src_loc[e]
        if sl < rA:
            lo_lists[r][b].append(src_core[e] * rA + sl)
            lo_dl[r][b].append(d_in_blk)
        else:
            hi_lists[r][b].append(src_core[e] * rB + (sl - rA))
            hi_dl[r][b].append(d_in_blk)

    if sort_src:
        # ascending table addresses within each block's chunk sequence ->
        # better HBM row-buffer / bank locality for the gather descriptors
        for r in range(n_cores):
            for b in range(nblk):
                for ids, dls in ((lo_lists[r][b], lo_dl[r][b]),
                                 (hi_lists[r][b], hi_dl[r][b])):
                    if ids:
                        pairs = sorted(zip(ids, dls))
                        ids[:] = [p[0] for p in pairs]
                        dls[:] = [p[1] for p in pairs]

    c_lo = max(
        1,
        max(
            math.ceil(len(lo_lists[r][b]) / P)
            for r in range(n_cores)
            for b in range(nblk)
        ),
    )
    n_hi = max(
        len(hi_lists[r][b]) for r in range(n_cores) for b in range(nblk)
    )
    c_hi = math.ceil(n_hi / P)  # may be 0
    cpb = c_lo + c_hi  # chunks per block

    # gather groups of up to `gb` blocks
    groups = []
    b0 = 0
    while b0 < nblk:
        g = min(gb, nblk - b0)
        groups.append((b0, g))
        b0 += g

    def wrap_idx(ids):
        """int16 wrap layout: idx i -> [i % 16, i // 16], replicated to 128
        partitions (8 groups of 16)."""
        ids = np.asarray(ids, dtype=np.int16)
        L = ids.shape[0]
        assert L % 16 == 0
        w = ids.reshape(L // 16, 16).T  # [16, L/16]
        return np.tile(w, (8, 1))  # [128, L/16]

    idx_lo = np.zeros((n_cores, P, nblk * c_lo * 8), dtype=np.int16)
    idx_hi = np.zeros((n_cores, P, max(1, nblk * c_hi * 8)), dtype=np.int16)
    dl_arr = np.full((n_cores, P, nblk * cpb), 300.0, dtype=np.float32)

    for r in range(n_cores):
        lo_col = 0
        hi_col = 0
        for b0, g in groups:
            lo_ids = []
            hi_ids = []
            for b in range(b0, b0 + g):
                ll = lo_lists[r][b]
                ll = ll + [0] * (c_lo * P - len(ll))
                lo_ids.extend(ll)
                hl = hi_lists[r][b]
                hl = hl + [0] * (c_hi * P - len(hl))
                hi_ids.extend(hl)
                # dst-local ids, chunk-major (lo chunks then hi chunks)
                dl_pad_lo = lo_dl[r][b] + [300] * (c_lo * P - len(lo_dl[r][b]))
                dl_pad_hi = hi_dl[r][b] + [300] * (c_hi * P - len(hi_dl[r][b]))
                dl_all = dl_pad_lo + dl_pad_hi
                for c in range(cpb):
                    dl_arr[r, :, b * cpb + c] = dl_all[c * P : (c + 1) * P]
            w = wrap_idx(lo_ids)
            idx_lo[r][:, lo_col : lo_col + w.shape[1]] = w
            lo_col += w.shape[1]
            if c_hi > 0:
                w = wrap_idx(hi_ids)
                idx_hi[r][:, hi_col : hi_col + w.shape[1]] = w
                hi_col += w.shape[1]

    dinv_own = np.zeros((n_cores, P, nblk), dtype=np.float32)
    for r in range(n_cores):
        own = dinv[r * npc : (r + 1) * npc]
        own = np.pad(own, (0, npc_pad - npc))
        dinv_own[r] = own.reshape(nblk, P).T

    return dict(
        n_cores=n_cores,
        N=N,
        E=E,
        nblkA=nblkA,
        rA=rA,
        rB=rB,
        npc=npc,
        nblk=nblk,
        npc_pad=npc_pad,
        c_lo=c_lo,
        c_hi=c_hi,
        groups=groups,
        idx_lo=idx_lo,
        idx_hi=idx_hi,
        dl=dl_arr,
        dinv_own=dinv_own,
    )


# ----------------------------------------------------------------------------
# Kernel builder (same BIR for all cores; per-core data via input tensors)
# ----------------------------------------------------------------------------
def build_kernel(plan, DIN, F1, F2, F3, skip=(), repeat=1, gbufs=3,
                 sbufs=4, tbufs=3):
    n_cores = plan["n_cores"]
    N = plan["N"]
    nblkA = plan["nblkA"]
    rA = plan["rA"]
    rB = plan["rB"]
    nblk = plan["nblk"]
    npc = plan["npc"]
    npc_pad = plan["npc_pad"]
    c_lo = plan["c_lo"]
    c_hi = plan["c_hi"]
    cpb = c_lo + c_hi
    groups = plan["groups"]
    rtA = n_cores * rA
    rtB = n_cores * rB
    rg = [list(range(n_cores))]

    nc = bacc.Bacc("TRN2", target_bir_lowering=False, debug=False,
                   num_devices=n_cores, num_swdge_queues=4)
    import itertools
    _gq = itertools.count()

    # ---- I/O ----
    aT0 = nc.dram_tensor("aT0", [P, npc_pad], BF16, kind="ExternalInput")
    W1 = nc.dram_tensor("W1", [P, F1], BF16, kind="ExternalInput")
    W2 = nc.dram_tensor("W2", [P, F2], BF16, kind="ExternalInput")
    W3 = nc.dram_tensor("W3", [P, F3], BF16, kind="ExternalInput")
    g1 = nc.dram_tensor("g1", [P, 1], F32, kind="ExternalInput")
    be1 = nc.dram_tensor("be1", [P, 1], F32, kind="ExternalInput")
    g2 = nc.dram_tensor("g2", [P, 1], F32, kind="ExternalInput")
    be2 = nc.dram_tensor("be2", [P, 1], F32, kind="ExternalInput")
    b3b = nc.dram_tensor("b3b", [P, F3], F32, kind="ExternalInput")
    iota_in = nc.dram_tensor("iota", [P, P], BF16, kind="ExternalInput")
    ident_in = nc.dram_tensor("ident", [P, P], BF16, kind="ExternalInput")
    idx_lo_in = nc.dram_tensor("idx_lo", list(plan["idx_lo"].shape[1:]), I16,
                               kind="ExternalInput")
    idx_hi_in = nc.dram_tensor("idx_hi", list(plan["idx_hi"].shape[1:]), I16,
                               kind="ExternalInput")
    dl_in = nc.dram_tensor("dl", [P, nblk * cpb], BF16, kind="ExternalInput")
    dinv_in = nc.dram_tensor("dinv_own", [P, nblk], F32, kind="ExternalInput")
    out_t = nc.dram_tensor("out", [npc, F3], F32, kind="ExternalOutput")

    with tile.TileContext(nc) as tc, ExitStack() as ctx:
        nc.gpsimd.load_library(library_config.mlp)

        sb = ctx.enter_context(tc.tile_pool(name="sb", bufs=1))
        # persistent sbuf state
        aT_a = sb.tile([P, npc_pad], BF16, tag="aT_a")
        aT_b = sb.tile([P, npc_pad], BF16, tag="aT_b")
        u_own = sb.tile([P, nblk, max(F1, F2)], BF16, tag="u_own")
        u_own3 = sb.tile([P, nblk, P], BF16, tag="u_own3")
        z_own = sb.tile([P, nblk, max(F1, F2)], BF16, tag="z_own")
        w_sb = sb.tile([P, F1 + F2 + F3], BF16, tag="w_sb")
        iota_t = sb.tile([P, P], BF16, tag="iota_t")
        ident_t = sb.tile([P, P], BF16, tag="ident_t")
        ones_t = sb.tile([P, 1], BF16, tag="ones_t")
        dinv_t = sb.tile([P, nblk], F32, tag="dinv_t")
        dl_t = sb.tile([P, nblk * cpb], BF16, tag="dl_t")
        ilo_t = sb.tile(list(plan["idx_lo"].shape[1:]), I16, tag="ilo_t")
        ihi_t = sb.tile(list(plan["idx_hi"].shape[1:]), I16, tag="ihi_t")
        bnp_t = sb.tile([P, 4], F32, tag="bnp_t")  # g1 be1 g2 be2
        b3_t = sb.tile([P, F3], F32, tag="b3_t")

        nc.sync.dma_start(w_sb[:, 0:F1], W1[:])
        nc.sync.dma_start(w_sb[:, F1:F1 + F2], W2[:])
        nc.sync.dma_start(w_sb[:, F1 + F2:], W3[:])
        nc.sync.dma_start(iota_t[:], iota_in[:])
        nc.sync.dma_start(ident_t[:], ident_in[:])
        nc.sync.dma_start(dinv_t[:], dinv_in[:])
        nc.sync.dma_start(dl_t[:], dl_in[:])
        nc.sync.dma_start(ilo_t[:], idx_lo_in[:])
        if c_hi > 0:
            nc.sync.dma_start(ihi_t[:], idx_hi_in[:])
        nc.sync.dma_start(bnp_t[:, 0:1], g1[:])
        nc.sync.dma_start(bnp_t[:, 1:2], be1[:])
        nc.sync.dma_start(bnp_t[:, 2:3], g2[:])
        nc.sync.dma_start(bnp_t[:, 3:4], be2[:])
        nc.sync.dma_start(b3_t[:], b3b[:])
        nc.sync.dma_start(aT_a[:], aT0[:])
        nc.gpsimd.memset(ones_t[:], 1.0)
        nc.gpsimd.memset(u_own3[:, :, F3:], 0.0)

        # DRAM scratch
        dram = ctx.enter_context(tc.tile_pool(name="dram", bufs=1,
                                              space="DRAM"))
        u1A = dram.tile([rA, F1], BF16, tag="u1A")
        u1B = dram.tile([rB, F1], BF16, tag="u1B")
        u2A = dram.tile([rA, F2], BF16, tag="u2A")
        u2B = dram.tile([rB, F2], BF16, tag="u2B")
        u3A = dram.tile([rA, P], BF16, tag="u3A")
        u3B = dram.tile([rB, P], BF16, tag="u3B")
        st_in1 = dram.tile([P, 2], F32, tag="st_in1")
        st_in2 = dram.tile([P, 2], F32, tag="st_in2")

        # working pools
        psum_mm = ctx.enter_context(
            tc.tile_pool(name="psum_mm", bufs=2, space="PSUM"))
        psum_agg = ctx.enter_context(
            tc.tile_pool(name="psum_agg", bufs=2, space="PSUM"))
        psum_st = ctx.enter_context(
            tc.tile_pool(name="psum_st", bufs=2, space="PSUM"))
        spool = ctx.enter_context(tc.tile_pool(name="spool", bufs=sbufs))
        gpool = ctx.enter_context(tc.tile_pool(name="gpool", bufs=gbufs))
        tpool = ctx.enter_context(tc.tile_pool(name="tpool", bufs=tbufs))

        gbmax = max(g for _, g in groups)

        def layer(l, aT_in, aT_out, F_in, F_out, w_off, udA, udB, ufA, ufB,
                  is_last, g_col=None, be_col=None, st_in=None, st_out=None):
            # ---------------- Phase A: dense matmul + u table ----------
            uo = u_own3 if is_last else u_own
            wtab = P if is_last else F_out

            def emit_half(ud, uf, b0, nb):
                nc.sync.dma_start(
                    ud[:].rearrange("(b p) f -> p b f", p=P),
                    uo[:, b0:b0 + nb, :wtab],
                )
                if n_cores > 1 and "coll" not in skip:
                    nc.gpsimd.collective_compute(
                        "AllGather", ALU.bypass, replica_groups=rg,
                        ins=[ud[:].opt()], outs=[uf[:].opt()],
                    )
                else:
                    nc.sync.dma_start(uf[0:nb * P, :], ud[:])

            for b in range(nblk):
                h_ps = psum_mm.tile([P, F_out], F32, tag="mm")
                nc.tensor.matmul(
                    h_ps[:],
                    lhsT=aT_in[:, b * P:(b + 1) * P],
                    rhs=w_sb[:, w_off:w_off + F_out],
                    start=True, stop=True,
                )
                nc.scalar.activation(uo[:, b, :F_out], h_ps[:], AF.Copy,
                                     scale=dinv_t[:, b:b + 1])
                if b == nblkA - 1:
                    emit_half(udA, ufA, 0, nblkA)
            emit_half(udB, ufB, nblkA, nblk - nblkA)

            # ---------------- Phase B: gather + segment matmul ---------
            if not is_last:
                st_s = psum_st.tile([P, 1], F32, tag="st_s")
                st_q = psum_st.tile([P, 1], F32, tag="st_q")
            lo_col = 0
            hi_col = 0
            for b0, g in groups:
                n_lo = g * c_lo * P
                lo_t = gpool.tile([P, gbmax * c_lo, P], BF16, tag="lo")
                if "noload" in skip:
                    pass
                elif "seqload" in skip:
                    nc.gpsimd.dma_start(
                        lo_t[:, :g * c_lo, :],
                        ufA[0:n_lo, :].rearrange(
                            "(c p) f -> p c f", p=P),
                    )
                else:
                    nc.gpsimd.dma_gather(
                        lo_t[:, :g * c_lo, :], ufA[:],
                        ilo_t[:, lo_col:lo_col + n_lo // 16],
                        n_lo, n_lo, P, single_packet=False,
                        queue_num=next(_gq) % 4,
                    )
                lo_col += n_lo // 16
                if c_hi > 0:
                    n_hi = g * c_hi * P
                    hi_t = gpool.tile([P, gbmax * c_hi, P], BF16, tag="hi")
                    if "noload" in skip:
                        pass
                    elif "seqload" in skip:
                        nc.gpsimd.dma_start(
                            hi_t[:, :g * c_hi, :],
                            ufB[0:n_hi, :].rearrange(
                                "(c p) f -> p c f", p=P),
                        )
                    else:
                        nc.gpsimd.dma_gather(
                            hi_t[:, :g * c_hi, :],
                            ufB[:],
                            ihi_t[:, hi_col:hi_col + n_hi // 16],
                            n_hi, n_hi, P, single_packet=False,
                            queue_num=next(_gq) % 4,
                        )
                    hi_col += n_hi // 16
                for bb in range(g):
                    b = b0 + bb
                    agg = psum_agg.tile([P, F_out], F32, tag="agg")
                    if "seg" in skip:
                        nc.vector.memset(agg[:], 0.0)
                    else:
                        s_w = spool.tile([P, cpb, P], BF16, tag="s")
                        nc.vector.tensor_tensor(
                            out=s_w[:],
                            in0=iota_t[:, None, :].to_broadcast([P, cpb, P]),
                            in1=dl_t[:, b * cpb:(b + 1) * cpb].to_broadcast(
                                [P, cpb, P]),
                            op=ALU.is_equal,
                        )
                        for c in range(cpb):
                            if c < c_lo:
                                rhs = lo_t[:, bb * c_lo + c, :F_out]
                            else:
                                rhs = hi_t[:, bb * c_hi + (c - c_lo), :F_out]
                            nc.tensor.matmul(
                                agg[:], lhsT=s_w[:, c, :], rhs=rhs,
                                start=(c == 0), stop=(c == cpb - 1),
                            )
                    # epilogue: z = dinv * (agg + u_own)
                    uo = u_own3 if is_last else u_own
                    t_t = tpool.tile([P, F_out], F32, tag="t")
                    nc.vector.tensor_tensor(
                        out=t_t[:], in0=agg[:], in1=uo[:, b, :F_out],
                        op=ALU.add,
                    )
                    if is_last:
                        z3 = tpool.tile([P, F_out], F32, tag="z3")
                        nc.scalar.activation(z3[:], t_t[:], AF.Copy,
                                             scale=dinv_t[:, b:b + 1])
                        o_t = tpool.tile([P, F_out], F32, tag="o")
                        nc.vector.tensor_tensor(out=o_t[:], in0=z3[:],
                                                in1=b3_t[:], op=ALU.add)
                        hi_row = min(npc, (b + 1) * P) - b * P
                        nc.sync.dma_start(out_t[b * P:b * P + hi_row, :],
                                          o_t[:hi_row, :])
                    else:
                        nc.scalar.activation(z_own[:, b, :F_out], t_t[:],
                                             AF.Copy,
                                             scale=dinv_t[:, b:b + 1])
                        z2 = tpool.tile([P, F_out], BF16, tag="z2")
                        nc.scalar.activation(z2[:], z_own[:, b, :F_out],
                                             AF.Square)
                        nc.tensor.matmul(st_s[:], lhsT=z_own[:, b, :F_out],
                                         rhs=ones_t[:],
                                         start=(b == 0), stop=(b == nblk - 1),
                                         skip_group_check=True)
                        nc.tensor.matmul(st_q[:], lhsT=z2[:], rhs=ones_t[:],
                                         start=(b == 0), stop=(b == nblk - 1),
                                         skip_group_check=True)
            if is_last:
                return

            # ---------------- Phase C: BN stats allreduce + coeffs -----
            st_sb = tpool.tile([P, 2], F32, tag="stsb")
            nc.vector.tensor_copy(st_sb[:, 0:1], st_s[:])
            nc.vector.tensor_copy(st_sb[:, 1:2], st_q[:])
            nc.sync.dma_start(st_in[:], st_sb[:])
            if n_cores > 1 and "coll" not in skip:
                nc.gpsimd.collective_compute(
                    "AllReduce", ALU.add, replica_groups=rg,
                    ins=[st_in[:].opt()], outs=[st_out[:].opt()],
                )
            else:
                nc.sync.dma_start(st_out[:], st_in[:])
            st_g = tpool.tile([P, 2], F32, tag="stg")
            nc.sync.dma_start(st_g[:], st_out[:])
            m_t = tpool.tile([P, 1], F32, tag="m")
            nc.scalar.activation(m_t[:], st_g[:, 0:1], AF.Copy, scale=1.0 / N)
            q_t = tpool.tile([P, 1], F32, tag="q")
            nc.scalar.activation(q_t[:], st_g[:, 1:2], AF.Copy, scale=1.0 / N)
            m2_t = tpool.tile([P, 1], F32, tag="m2")
            nc.scalar.activation(m2_t[:], m_t[:], AF.Square)
            v_t = tpool.tile([P, 1], F32, tag="v")
            nc.vector.tensor_tensor(out=v_t[:], in0=q_t[:], in1=m2_t[:],
                                    op=ALU.subtract)
            ve_t = tpool.tile([P, 1], F32, tag="ve")
            nc.vector.tensor_scalar(out=ve_t[:], in0=v_t[:], scalar1=1e-5,
                                    scalar2=None, op0=ALU.add)
            sd_t = tpool.tile([P, 1], F32, tag="sd")
            nc.scalar.activation(sd_t[:], ve_t[:], AF.Sqrt)
            inv_t = tpool.tile([P, 1], F32, tag="inv")
            nc.vector.reciprocal(inv_t[:], sd_t[:])
            a_t = tpool.tile([P, 1], F32, tag="A")
            nc.vector.tensor_tensor(out=a_t[:], in0=bnp_t[:, g_col:g_col + 1],
                                    in1=inv_t[:], op=ALU.mult)
            ma_t = tpool.tile([P, 1], F32, tag="mA")
            nc.vector.tensor_tensor(out=ma_t[:], in0=m_t[:], in1=a_t[:],
                                    op=ALU.mult)
            bb_t = tpool.tile([P, 1], F32, tag="B")
            nc.vector.tensor_tensor(out=bb_t[:],
                                    in0=bnp_t[:, be_col:be_col + 1],
                                    in1=ma_t[:], op=ALU.subtract)

            # ---------------- Phase D: transpose + BN apply + relu -----
            for b in range(nblk):
                zT = psum_mm.tile([P, P], BF16, tag="mm")
                nc.tensor.transpose(zT[:], z_own[:, b, :F_out], ident_t[:])
                nc.scalar.activation(aT_out[:, b * P:(b + 1) * P], zT[:],
                                     AF.Relu, bias=bb_t[:], scale=a_t[:])

        for _rep in range(repeat):
            uf1A = dram.tile([rtA, F1], BF16, tag=f"uf1A_{_rep}",
                             addr_space="Shared")
            uf1B = dram.tile([rtB, F1], BF16, tag=f"uf1B_{_rep}",
                             addr_space="Shared")
            uf2A = dram.tile([rtA, F2], BF16, tag=f"uf2A_{_rep}",
                             addr_space="Shared")
            uf2B = dram.tile([rtB, F2], BF16, tag=f"uf2B_{_rep}",
                             addr_space="Shared")
            uf3A = dram.tile([rtA, P], BF16, tag=f"uf3A_{_rep}",
                             addr_space="Shared")
            uf3B = dram.tile([rtB, P], BF16, tag=f"uf3B_{_rep}",
                             addr_space="Shared")
            st_out1 = dram.tile([P, 2], F32, tag=f"st_out1_{_rep}",
                                addr_space="Shared")
            st_out2 = dram.tile([P, 2], F32, tag=f"st_out2_{_rep}",
                                addr_space="Shared")
            if _rep > 0:
                nc.sync.dma_start(aT_a[:], aT0[:])
            layer(1, aT_a, aT_b, DIN, F1, 0, u1A, u1B, uf1A, uf1B, False,
                  0, 1, st_in1, st_out1)
            layer(2, aT_b, aT_a, F1, F2, F1, u2A, u2B, uf2A, uf2B, False,
                  2, 3, st_in2, st_out2)
            layer(3, aT_a, None, F2, F3, F1 + F2, u3A, u3B, uf3A, uf3B,
                  True)

    nc.compile()
    return nc


# ----------------------------------------------------------------------------
# Kernel v2: aggregate-first ordering  out = (A_hat @ z) @ W
#   - the gathered table holds y = dinv * z (node-major rows, bf16); layer 1
#     gathers straight from the replicated input tables (no AllGather);
#     layer 3 needs no table at all -> only 2 table AllGather pairs total.
#   - the self-loop term enters the PSUM accumulation as one identity matmul
#     of the local y block (no extra gather/transpose/add).
#   - segment matmul emits aggT feature-major, which feeds the dense matmul
#     as lhsT directly; BN+ReLU applied node-major on DVE with exact f32
#     per-feature rows (block-transpose + partition_broadcast).
# ----------------------------------------------------------------------------
def build_kernel2(plan, DIN, F1, F2, F3, skip=(), repeat=1, gbufs=4,
                  sbufs=4, tbufs=6, abufs=2, mbufs=2, dbg=(),
                  single_packet=False, nqueues=4):
    n_cores = plan["n_cores"]
    N = plan["N"]
    nblkA = plan["nblkA"]
    rA = plan["rA"]
    rB = plan["rB"]
    nblk = plan["nblk"]
    npc = plan["npc"]
    c_lo = plan["c_lo"]
    c_hi = plan["c_hi"]
    cpb = c_lo + c_hi
    groups = plan["groups"]
    rtA = n_cores * rA
    rtB = n_cores * rB
    rg = [list(range(n_cores))]
    assert DIN == 128 and F1 == 128 and F2 == 128

    nc = bacc.Bacc("TRN2", target_bir_lowering=False, debug=False,
                   num_devices=n_cores, num_swdge_queues=nqueues)
    import itertools
    _gq = itertools.count()

    # ---- I/O ----
    tA0 = nc.dram_tensor("tA0", [rtA, DIN], BF16, kind="ExternalInput")
    tB0 = nc.dram_tensor("tB0", [rtB, DIN], BF16, kind="ExternalInput")
    yown0 = nc.dram_tensor("yown0", [P, nblk * DIN], BF16,
                           kind="ExternalInput")
    W1 = nc.dram_tensor("W1", [P, F1], BF16, kind="ExternalInput")
    W2 = nc.dram_tensor("W2", [P, F2], BF16, kind="ExternalInput")
    W3 = nc.dram_tensor("W3", [P, F3], BF16, kind="ExternalInput")
    g1 = nc.dram_tensor("g1", [P, 1], F32, kind="ExternalInput")
    be1 = nc.dram_tensor("be1", [P, 1], F32, kind="ExternalInput")
    g2 = nc.dram_tensor("g2", [P, 1], F32, kind="ExternalInput")
    be2 = nc.dram_tensor("be2", [P, 1], F32, kind="ExternalInput")
    b3b = nc.dram_tensor("b3b", [P, F3], F32, kind="ExternalInput")
    iota_in = nc.dram_tensor("iota", [P, P], BF16, kind="ExternalInput")
    ident_in = nc.dram_tensor("ident", [P, P], BF16, kind="ExternalInput")
    idx_lo_in = nc.dram_tensor("idx_lo", list(plan["idx_lo"].shape[1:]), I16,
                               kind="ExternalInput")
    idx_hi_in = nc.dram_tensor("idx_hi", list(plan["idx_hi"].shape[1:]), I16,
                               kind="ExternalInput")
    dl_in = nc.dram_tensor("dl", [P, nblk * cpb], BF16, kind="ExternalInput")
    dinv_in = nc.dram_tensor("dinv_own", [P, nblk], F32, kind="ExternalInput")
    out_t = nc.dram_tensor("out", [npc, F3], F32, kind="ExternalOutput")
    dbg_t = {}
    for d in dbg:
        if d in ("u1", "y1"):
            dbg_t[d] = nc.dram_tensor(f"dbg_{d}", [P, nblk * 128], BF16,
                                      kind="ExternalOutput")
        else:
            dbg_t[d] = nc.dram_tensor(f"dbg_{d}", [P, 128], F32,
                                      kind="ExternalOutput")

    with tile.TileContext(nc) as tc, ExitStack() as ctx:
        nc.gpsimd.load_library(library_config.mlp)

        sb = ctx.enter_context(tc.tile_pool(name="sb", bufs=1))
        w_sb = sb.tile([P, F1 + F2 + F3], BF16, tag="w_sb")
        iota_t = sb.tile([P, P], BF16, tag="iota_t")
        ident_t = sb.tile([P, P], BF16, tag="ident_t")
        ones_t = sb.tile([P, 1], BF16, tag="ones_t")
        ones_r = sb.tile([1, P], BF16, tag="ones_r")
        dinv_t = sb.tile([P, nblk], F32, tag="dinv_t")
        dl_t = sb.tile([P, nblk * cpb], BF16, tag="dl_t")
        ilo_t = sb.tile(list(plan["idx_lo"].shape[1:]), I16, tag="ilo_t")
        ihi_t = sb.tile(list(plan["idx_hi"].shape[1:]), I16, tag="ihi_t")
        bnp_t = sb.tile([P, 4], F32, tag="bnp_t")
        b3_t = sb.tile([P, F3], F32, tag="b3_t")
        y_a = sb.tile([P, nblk, 128], BF16, tag="y_a")
        y_b = sb.tile([P, nblk, 128], BF16, tag="y_b")
        u_keep = sb.tile([P, nblk, 128], BF16, tag="u_keep")

        nc.sync.dma_start(w_sb[:, 0:F1], W1[:])
        nc.sync.dma_start(w_sb[:, F1:F1 + F2], W2[:])
        nc.sync.dma_start(w_sb[:, F1 + F2:], W3[:])
        nc.sync.dma_start(iota_t[:], iota_in[:])
        nc.sync.dma_start(ident_t[:], ident_in[:])
        nc.sync.dma_start(dinv_t[:], dinv_in[:])
        nc.sync.dma_start(dl_t[:], dl_in[:])
        nc.sync.dma_start(ilo_t[:], idx_lo_in[:])
        if c_hi > 0:
            nc.sync.dma_start(ihi_t[:], idx_hi_in[:])
        nc.sync.dma_start(bnp_t[:, 0:1], g1[:])
        nc.sync.dma_start(bnp_t[:, 1:2], be1[:])
        nc.sync.dma_start(bnp_t[:, 2:3], g2[:])
        nc.sync.dma_start(bnp_t[:, 3:4], be2[:])
        nc.sync.dma_start(b3_t[:], b3b[:])
        nc.sync.dma_start(
            y_a[:], yown0[:].rearrange("p (b f) -> p b f", b=nblk))
        nc.gpsimd.memset(ones_t[:], 1.0)
        nc.gpsimd.memset(ones_r[:], 1.0)

        dram = ctx.enter_context(tc.tile_pool(name="dram", bufs=1,
                                              space="DRAM"))
        d1A = dram.tile([rA, F1], BF16, tag="d1A")
        d1B = dram.tile([rB, F1], BF16, tag="d1B")
        d2A = dram.tile([rA, F2], BF16, tag="d2A")
        d2B = dram.tile([rB, F2], BF16, tag="d2B")
        st_in1 = dram.tile([P, 2], F32, tag="st_in1")
        st_in2 = dram.tile([P, 2], F32, tag="st_in2")

        psum_agg = ctx.enter_context(
            tc.tile_pool(name="psum_agg", bufs=abufs, space="PSUM"))
        psum_mm = ctx.enter_context(
            tc.tile_pool(name="psum_mm", bufs=mbufs, space="PSUM"))
        psum_st = ctx.enter_context(
            tc.tile_pool(name="psum_st", bufs=2, space="PSUM"))
        spool = ctx.enter_context(tc.tile_pool(name="spool", bufs=sbufs))
        gpool = ctx.enter_context(tc.tile_pool(name="gpool", bufs=gbufs))
        tpool = ctx.enter_context(tc.tile_pool(name="tpool", bufs=tbufs))

        gbmax = max(g for _, g in groups)

        def emit_half(y_src, ud, uf, b0, nb, F_out):
            nc.sync.dma_start(
                ud[:].rearrange("(b p) f -> p b f", p=P),
                y_src[:, b0:b0 + nb, :F_out],
            )
            if n_cores > 1 and "coll" not in skip:
                nc.gpsimd.collective_compute(
                    "AllGather", ALU.bypass, replica_groups=rg,
                    ins=[ud[:].opt()], outs=[uf[:].opt()],
                )
            else:
                nc.sync.dma_start(uf[0:nb * P, :], ud[:])

        def layer2(l, y_cur, y_nxt, F_out, w_rhs, tfA, tfB, udA=None,
                   udB=None, tfA_out=None, tfB_out=None, g_col=None,
                   be_col=None, st_in=None, st_out=None, w_next=None):
            """One GCN layer, aggregate-first.  w_rhs: SBUF [128, F_out]
            effective weight (previous layer's BN scale pre-folded).
            w_next: SBUF slice of the NEXT layer's raw weight; returns its
            A-scaled copy for the next layer2 call."""
            last = (l == 3)
            if not last:
                st_s = psum_st.tile([P, 1], F32, tag="st_s", bufs=1)
                st_q = psum_st.tile([P, 1], F32, tag="st_q", bufs=1)
            d512 = "d512" in skip  # timing diagnostic: half descs, 512B
            cd = lambda n: (n + 1) // 2
            lo_col = 0
            hi_col = 0
            for b0, g in groups:
                n_lo = g * c_lo * P
                if d512:
                    lo_t = gpool.tile([P, cd(gbmax * c_lo), 2 * P], BF16,
                                      tag="lo")
                    nc.gpsimd.dma_gather(
                        lo_t[:, :cd(g * c_lo), :],
                        tfA[:].rearrange("(r two) f -> r (two f)", two=2),
                        ilo_t[:, lo_col:lo_col + (n_lo // 2) // 16],
                        n_lo // 2, n_lo // 2, 2 * P,
                        single_packet=single_packet,
                        queue_num=next(_gq) % nqueues,
                    )
                else:
                    lo_t = gpool.tile([P, gbmax * c_lo, P], BF16, tag="lo")
                    if "gsm" not in skip:
                        nc.gpsimd.dma_gather(
                            lo_t[:, :g * c_lo, :], tfA[:],
                            ilo_t[:, lo_col:lo_col + n_lo // 16],
                            n_lo, n_lo, P, single_packet=single_packet,
                            queue_num=next(_gq) % nqueues,
                        )
                lo_col += n_lo // 16
                if c_hi > 0:
                    n_hi = g * c_hi * P
                    if d512:
                        hi_t = gpool.tile([P, cd(gbmax * c_hi), 2 * P], BF16,
                                          tag="hi")
                        nc.gpsimd.dma_gather(
                            hi_t[:, :cd(g * c_hi), :],
                            tfB[:].rearrange("(r two) f -> r (two f)", two=2),
                            ihi_t[:, hi_col:hi_col + (n_hi // 2) // 16],
                            n_hi // 2, n_hi // 2, 2 * P,
                            single_packet=single_packet,
                            queue_num=next(_gq) % nqueues,
                        )
                    else:
                        hi_t = gpool.tile([P, gbmax * c_hi, P], BF16,
                                          tag="hi")
                        if "gsm" not in skip:
                            nc.gpsimd.dma_gather(
                                hi_t[:, :g * c_hi, :], tfB[:],
                                ihi_t[:, hi_col:hi_col + n_hi // 16],
                                n_hi, n_hi, P, single_packet=single_packet,
                                queue_num=next(_gq) % nqueues,
                            )
                    hi_col += n_hi // 16
                for bb in range(g):
                    b = b0 + bb
                    agg = psum_agg.tile([P, P], F32, tag="agg")
                    if "gsm" in skip:
                        nc.tensor.matmul(agg[:], lhsT=y_cur[:, b, :],
                                         rhs=ident_t[:], start=True,
                                         stop=True)
                    else:
                        s_w = spool.tile([P, cpb, P], BF16, tag="s")
                        nc.vector.tensor_tensor(
                            out=s_w[:],
                            in0=iota_t[:, None, :].to_broadcast([P, cpb, P]),
                            in1=dl_t[:, b * cpb:(b + 1) * cpb].to_broadcast(
                                [P, cpb, P]),
                            op=ALU.is_equal,
                        )
                        nc.tensor.matmul(agg[:], lhsT=y_cur[:, b, :],
                                         rhs=ident_t[:], start=True,
                                         stop=False)
                        for c in range(cpb):
                            if c < c_lo:
                                i, t = bb * c_lo + c, lo_t
                            else:
                                i, t = bb * c_hi + (c - c_lo), hi_t
                            if d512:
                                lhs = t[:, i // 2,
                                        (i % 2) * P:(i % 2 + 1) * P]
                            else:
                                lhs = t[:, i, :]
                            nc.tensor.matmul(agg[:], lhsT=lhs,
                                             rhs=s_w[:, c, :],
                                             start=False, stop=(c == cpb - 1))
                    agg_sb = tpool.tile([P, P], BF16, tag="aggsb")
                    nc.scalar.activation(agg_sb[:], agg[:], AF.Copy)
                    u_ps = psum_mm.tile([P, F_out], F32, tag="u")
                    nc.tensor.matmul(u_ps[:], lhsT=agg_sb[:], rhs=w_rhs,
                                     start=True, stop=True)
                    if last:
                        o_t = tpool.tile([P, F_out], F32, tag="o")
                        nc.scalar.activation(o_t[:], u_ps[:], AF.Copy,
                                             scale=dinv_t[:, b:b + 1])
                        o2 = tpool.tile([P, F_out], F32, tag="o2")
                        nc.vector.tensor_tensor(out=o2[:], in0=o_t[:],
                                                in1=b3_t[:], op=ALU.add)
                        hi_row = min(npc, (b + 1) * P) - b * P
                        nc.sync.dma_start(out_t[b * P:b * P + hi_row, :],
                                          o2[:hi_row, :])
                    else:
                        nc.scalar.activation(u_keep[:, b, :F_out], u_ps[:],
                                             AF.Copy, scale=dinv_t[:, b:b + 1])
                        u2 = tpool.tile([P, F_out], BF16, tag="u2")
                        nc.scalar.activation(u2[:], u_keep[:, b, :F_out],
                                             AF.Square)
                        nc.tensor.matmul(st_s[:], lhsT=u_keep[:, b, :F_out],
                                         rhs=ones_t[:], start=(b == 0),
                                         stop=(b == nblk - 1),
                                         skip_group_check=True)
                        nc.tensor.matmul(st_q[:], lhsT=u2[:], rhs=ones_t[:],
                                         start=(b == 0), stop=(b == nblk - 1),
                                         skip_group_check=True)
            if last:
                return

            if l == 1 and "u1" in dbg_t:
                nc.sync.dma_start(
                    dbg_t["u1"][:].rearrange("p (b f) -> p b f", b=nblk),
                    u_keep[:])

            # BN stats allreduce + coefficient columns
            st_sb = tpool.tile([P, 2], F32, tag="stsb")
            nc.vector.tensor_copy(st_sb[:, 0:1], st_s[:])
            nc.vector.tensor_copy(st_sb[:, 1:2], st_q[:])
            nc.sync.dma_start(st_in[:], st_sb[:])
            if n_cores > 1 and "coll" not in skip:
                nc.gpsimd.collective_compute(
                    "AllReduce", ALU.add, replica_groups=rg,
                    ins=[st_in[:].opt()], outs=[st_out[:].opt()],
                )
            else:
                nc.sync.dma_start(st_out[:], st_in[:])
            st_g = tpool.tile([P, 2], F32, tag="stg")
            nc.sync.dma_start(st_g[:], st_out[:])
            m_t = tpool.tile([P, 1], F32, tag="m")
            nc.scalar.activation(m_t[:], st_g[:, 0:1], AF.Copy, scale=1.0 / N)
            q_t = tpool.tile([P, 1], F32, tag="q")
            nc.scalar.activation(q_t[:], st_g[:, 1:2], AF.Copy, scale=1.0 / N)
            m2_t = tpool.tile([P, 1], F32, tag="m2")
            nc.scalar.activation(m2_t[:], m_t[:], AF.Square)
            v_t = tpool.tile([P, 1], F32, tag="v")
            nc.vector.tensor_tensor(out=v_t[:], in0=q_t[:], in1=m2_t[:],
                                    op=ALU.subtract)
            ve_t = tpool.tile([P, 1], F32, tag="ve")
            nc.vector.tensor_scalar(out=ve_t[:], in0=v_t[:], scalar1=1e-5,
                                    scalar2=None, op0=ALU.add)
            sd_t = tpool.tile([P, 1], F32, tag="sd")
            nc.scalar.activation(sd_t[:], ve_t[:], AF.Sqrt)
            inv_t = tpool.tile([P, 1], F32, tag="inv")
            nc.vector.reciprocal(inv_t[:], sd_t[:])
            a_t = tpool.tile([P, 1], F32, tag="A")
            nc.vector.tensor_tensor(out=a_t[:], in0=bnp_t[:, g_col:g_col + 1],
                                    in1=inv_t[:], op=ALU.mult)
            ma_t = tpool.tile([P, 1], F32, tag="mA")
            nc.vector.tensor_tensor(out=ma_t[:], in0=m_t[:], in1=a_t[:],
                                    op=ALU.mult)
            bb_t = tpool.tile([P, 1], F32, tag="B")
            nc.vector.tensor_tensor(out=bb_t[:],
                                    in0=bnp_t[:, be_col:be_col + 1],
                                    in1=ma_t[:], op=ALU.subtract)

            # BN rewrite for A>0:  z = A*u + B -> relu(z) = A * relu(u + C),
            # C = B/A.  The table stores y~ = dinv * relu(u + C) (per-feature
            # scale A folded into the NEXT layer's weight rows, exactly);
            # the bias C is broadcast to a full [128,128] tile via two PE
            # matmuls (column -> row -> rank-1 broadcast).
            rc_t = tpool.tile([P, 1], F32, tag="rc")
            nc.vector.reciprocal(rc_t[:], a_t[:])
            c_t = tpool.tile([P, 1], F32, tag="c")
            nc.vector.tensor_tensor(out=c_t[:], in0=bb_t[:], in1=rc_t[:],
                                    op=ALU.mult)
            c_bf = tpool.tile([P, 1], BF16, tag="cbf")
            nc.scalar.activation(c_bf[:], c_t[:], AF.Copy)
            crow_ps = psum_st.tile([1, P], F32, tag="crow", bufs=1)
            nc.tensor.matmul(crow_ps[:], lhsT=c_bf[:], rhs=ident_t[:],
                             start=True, stop=True)
            crow_sb = tpool.tile([1, P], BF16, tag="crowsb")
            nc.scalar.activation(crow_sb[:], crow_ps[:], AF.Copy)
            cf_ps = psum_st.tile([P, P], F32, tag="cf", bufs=1)
            nc.tensor.matmul(cf_ps[:], lhsT=ones_r[:], rhs=crow_sb[:],
                             start=True, stop=True)
            C_full = tpool.tile([P, P], F32, tag="Cfull")
            nc.scalar.activation(C_full[:], cf_ps[:], AF.Copy)
            # fold A into the next layer's weight rows (exact, f32 scale)
            w_eff = sb.tile([P, w_next.shape[-1]], BF16, tag=f"weff{l}")
            nc.scalar.activation(w_eff[:], w_next, AF.Copy, scale=a_t[:])

            if l == 1 and "st1" in dbg_t:
                stdbg = tpool.tile([P, 128], F32, tag="stdbg")
                nc.vector.memset(stdbg[:], 0.0)
                nc.vector.tensor_copy(stdbg[:, 0:2], st_g[:])
                nc.vector.tensor_copy(stdbg[:, 2:3], a_t[:])
                nc.vector.tensor_copy(stdbg[:, 3:4], bb_t[:])
                nc.sync.dma_start(dbg_t["st1"][:], stdbg[:])
            if l == 1 and "af1" in dbg_t:
                nc.sync.dma_start(dbg_t["af1"][:], C_full[:])

            # bias + relu + dinv pre-scale, node-major; emit halves
            for b in range(nblk):
                t2 = tpool.tile([P, F_out], F32, tag="t2")
                nc.vector.tensor_tensor(out=t2[:], in0=u_keep[:, b, :F_out],
                                        in1=C_full[:, :F_out], op=ALU.add)
                nc.scalar.activation(y_nxt[:, b, :F_out], t2[:], AF.Relu,
                                     scale=dinv_t[:, b:b + 1])
                if b == nblkA - 1:
                    emit_half(y_nxt, udA, tfA_out, 0, nblkA, F_out)
            emit_half(y_nxt, udB, tfB_out, nblkA, nblk - nblkA, F_out)
            if l == 1 and "y1" in dbg_t:
                nc.sync.dma_start(
                    dbg_t["y1"][:].rearrange("p (b f) -> p b f", b=nblk),
                    y_nxt[:])
            return w_eff

        for _rep in range(repeat):
            tf1A = dram.tile([rtA, F1], BF16, tag=f"tf1A_{_rep}",
                             addr_space="Shared")
            tf1B = dram.tile([rtB, F1], BF16, tag=f"tf1B_{_rep}",
                             addr_space="Shared")
            tf2A = dram.tile([rtA, F2], BF16, tag=f"tf2A_{_rep}",
                             addr_space="Shared")
            tf2B = dram.tile([rtB, F2], BF16, tag=f"tf2B_{_rep}",
                             addr_space="Shared")
            st_out1 = dram.tile([P, 2], F32, tag=f"st_out1_{_rep}",
                                addr_space="Shared")
            st_out2 = dram.tile([P, 2], F32, tag=f"st_out2_{_rep}",
                                addr_space="Shared")
            if _rep > 0:
                nc.sync.dma_start(
                    y_a[:], yown0[:].rearrange("p (b f) -> p b f", b=nblk))
            w2_eff = layer2(1, y_a, y_b, F1, w_sb[:, 0:F1], tA0, tB0,
                            d1A, d1B, tf1A, tf1B, 0, 1, st_in1, st_out1,
                            w_next=w_sb[:, F1:F1 + F2])
            w3_eff = layer2(2, y_b, y_a, F2, w2_eff[:], tf1A, tf1B,
                            d2A, d2B, tf2A, tf2B, 2, 3, st_in2, st_out2,
                            w_next=w_sb[:, F1 + F2:F1 + F2 + F3])
            layer2(3, y_a, None, F3, w3_eff[:], tf2A, tf2B)

    nc.compile()
    return nc


def make_in_maps2(plan, inputs, DIN, F1, F2, F3):
    n_cores = plan["n_cores"]
    npc = plan["npc"]
    npc_pad = plan["npc_pad"]
    nblk = plan["nblk"]
    rA = plan["rA"]
    rB = plan["rB"]
    N = plan["N"]
    x = np.asarray(inputs["x"], dtype=np.float32)
    edge_index = np.asarray(inputs["edge_index"])
    import ml_dtypes
    bf16 = ml_dtypes.bfloat16

    dst = np.asarray(edge_index[1], dtype=np.int64)
    deg = np.bincount(dst, minlength=N).astype(np.float64) + 1.0
    dinv = (1.0 / np.sqrt(deg)).astype(np.float32)
    y0 = (x * dinv[:, None]).astype(bf16)

    tA0 = np.zeros((n_cores * rA, DIN), bf16)
    tB0 = np.zeros((n_cores * rB, DIN), bf16)
    yown0 = np.zeros((n_cores, P, nblk * DIN), bf16)
    for c in range(n_cores):
        yc = np.zeros((npc_pad, DIN), bf16)
        yc[:npc] = y0[c * npc:(c + 1) * npc]
        tA0[c * rA:(c + 1) * rA] = yc[:rA]
        tB0[c * rB:(c + 1) * rB] = yc[rA:]
        yown0[c] = yc.reshape(nblk, P, DIN).transpose(1, 0, 2).reshape(
            P, nblk * DIN)

    iota = np.tile(np.arange(P)[None, :], (P, 1)).astype(bf16)
    ident = np.eye(P, dtype=bf16)
    b3b = np.tile(np.asarray(inputs["b3"], np.float32)[None, :], (P, 1))
    col = lambda v: np.asarray(v, np.float32).reshape(P, 1)
    in_maps = []
    for r in range(n_cores):
        in_maps.append({
            "tA0": tA0, "tB0": tB0, "yown0": yown0[r],
            "W1": np.asarray(inputs["W1"], np.float32).astype(bf16),
            "W2": np.asarray(inputs["W2"], np.float32).astype(bf16),
            "W3": np.asarray(inputs["W3"], np.float32).astype(bf16),
            "g1": col(inputs["g1"]), "be1": col(inputs["be1"]),
            "g2": col(inputs["g2"]), "be2": col(inputs["be2"]),
            "b3b": b3b, "iota": iota, "ident": ident,
            "idx_lo": plan["idx_lo"][r], "idx_hi": plan["idx_hi"][r],
            "dl": plan["dl"][r].astype(bf16),
            "dinv_own": plan["dinv_own"][r],
        })
    return in_maps


# ----------------------------------------------------------------------------
# Host entry point
# ----------------------------------------------------------------------------
def make_in_maps(plan, inputs, DIN, F1, F2, F3):
    n_cores = plan["n_cores"]
    npc = plan["npc"]
    npc_pad = plan["npc_pad"]
    x = np.asarray(inputs["x"], dtype=np.float32)
    import ml_dtypes
    bf16 = ml_dtypes.bfloat16
    iota = np.tile(np.arange(P)[None, :], (P, 1)).astype(bf16)
    ident = np.eye(P, dtype=bf16)
    b3b = np.tile(np.asarray(inputs["b3"], np.float32)[None, :], (P, 1))
    col = lambda v: np.asarray(v, np.float32).reshape(P, 1)
    in_maps = []
    for r in range(n_cores):
        xr = x[r * npc:(r + 1) * npc]
        aT0 = np.zeros((P, npc_pad), bf16)
        aT0[:, :npc] = xr.T.astype(bf16)
        in_maps.append({
            "aT0": aT0,
            "W1": np.asarray(inputs["W1"], np.float32).astype(bf16),
            "W2": np.asarray(inputs["W2"], np.float32).astype(bf16),
            "W3": np.asarray(inputs["W3"], np.float32).astype(bf16),
            "g1": col(inputs["g1"]), "be1": col(inputs["be1"]),
            "g2": col(inputs["g2"]), "be2": col(inputs["be2"]),
            "b3b": b3b, "iota": iota, "ident": ident,
            "idx_lo": plan["idx_lo"][r], "idx_hi": plan["idx_hi"][r],
            "dl": plan["dl"][r].astype(ml_dtypes.bfloat16), "dinv_own": plan["dinv_own"][r],
        })
    return in_maps


_CACHE = {}


def _sharded_runner(nc, in_maps):
    """Build a single jit/shard_map executable for `nc` (same lowering path
    run_bass_kernel_spmd uses under axon) and return
    (call(dev_in, dev_zeros) -> out_arrs, stage() -> (dev_in, dev_zeros),
    out_names, out_avals)."""
    import jax
    from jax.sharding import Mesh, PartitionSpec, NamedSharding
    from jax.experimental.shard_map import shard_map
    from concourse.bass2jax import (
        _bass_exec_p, install_neuronx_cc_hook, partition_id_tensor)

    install_neuronx_cc_hook()
    n_cores = len(in_maps)
    partition_name = (
        nc.partition_id_tensor.name if nc.partition_id_tensor else None)
    in_names, out_names, out_avals, zero_outs = [], [], [], []
    for alloc in nc.m.functions[0].allocations:
        if not isinstance(alloc, mybir.MemoryLocationSet):
            continue
        name = alloc.memorylocations[0].name
        if alloc.kind == "ExternalInput":
            if name != partition_name:
                in_names.append(name)
        elif alloc.kind == "ExternalOutput":
            out_names.append(name)
            shape = tuple(alloc.tensor_shape)
            dtype = mybir.dt.np(alloc.dtype)
            out_avals.append(jax.core.ShapedArray(shape, dtype))
            zero_outs.append(np.zeros(shape, dtype))
    n_params = len(in_names)
    n_outs = len(out_avals)
    all_in_names = list(in_names) + list(out_names)
    if partition_name is not None:
        all_in_names.append(partition_name)

    def _body(*args):
        operands = list(args)
        if partition_name is not None:
            operands.append(partition_id_tensor())
        outs = _bass_exec_p.bind(
            *operands,
            out_avals=tuple(out_avals),
            in_names=tuple(all_in_names),
            out_names=tuple(out_names),
            lowering_input_output_aliases=(),
            sim_require_finite=True,
            sim_require_nnan=True,
            nc=nc,
        )
        return tuple(outs)

    devices = jax.devices()[:n_cores]
    mesh = Mesh(np.asarray(devices), ("core",))
    donate = tuple(range(n_params, n_params + n_outs))
    sharded = jax.jit(
        shard_map(_body, mesh=mesh,
                  in_specs=(PartitionSpec("core"),) * (n_params + n_outs),
                  out_specs=(PartitionSpec("core"),) * n_outs,
                  check_rep=False),
        donate_argnums=donate, keep_unused=True)
    sharding = NamedSharding(mesh, PartitionSpec("core"))
    per_core = [[np.asarray(m[name]) for name in in_names] for m in in_maps]
    concat_in = [
        np.concatenate([per_core[c][i] for c in range(n_cores)], axis=0)
        for i in range(n_params)]
    concat_zeros = [
        np.zeros((n_cores * z.shape[0], *z.shape[1:]), z.dtype)
        for z in zero_outs]

    def stage_in():
        dev_in = [jax.device_put(a, sharding) for a in concat_in]
        jax.block_until_ready(dev_in)
        return dev_in

    def stage_zeros():
        dev_zeros = [jax.device_put(z, sharding) for z in concat_zeros]
        jax.block_until_ready(dev_zeros)
        return dev_zeros

    return sharded, stage_in, stage_zeros, out_names, out_avals


V2 = True  # use the aggregate-first kernel (build_kernel2)
REPK = 8   # marginal timing reference: repeat=REPK vs repeat=1


def run_timed(inputs, rounds=10, iters_per_round=5):
    """Measure the kernel's on-device execution time via the marginal-wall
    method: build the network once (repeat=1) and twice back-to-back
    (repeat=2) in one NEFF each, time both executables with inputs staged on
    device (donated zero output buffers staged untimed per call), and report
    best2 - best1 per adjacent round (robust to slow drift in the axon
    dispatch-floor, which is ~50-130x the kernel time and excluded by the
    subtraction).  The NTFF profiling path is unavailable under this axon
    client; this is the closest honest proxy for the HW execution span.

    Returns (full output ndarray, marginal seconds, walls1, walls2)."""
    import time
    import jax

    x = np.asarray(inputs["x"], dtype=np.float32)
    N, DIN = x.shape
    F1 = inputs["W1"].shape[1]
    F2 = inputs["W2"].shape[1]
    F3 = inputs["W3"].shape[1]
    edge_index = np.asarray(inputs["edge_index"])
    plan = make_plan(edge_index, N, sort_src=V2)
    build = build_kernel2 if V2 else build_kernel
    mkmaps = make_in_maps2 if V2 else make_in_maps
    nc1 = build(plan, DIN, F1, F2, F3, repeat=1)
    nc2 = build(plan, DIN, F1, F2, F3, repeat=REPK)
    in_maps = mkmaps(plan, inputs, DIN, F1, F2, F3)
    n_cores = plan["n_cores"]

    class R:
        def __init__(self, nc):
            (self.sharded, stage_in, self.stage_zeros, self.out_names,
             self.out_avals) = _sharded_runner(nc, in_maps)
            self.dev_in = stage_in()
            self.last_out = None

    def timed_calls(r, n):
        ws = []
        for _ in range(n):
            dev_zeros = r.stage_zeros()
            t0 = time.perf_counter()
            oa = r.sharded(*r.dev_in, *dev_zeros)
            jax.block_until_ready(oa)
            ws.append(time.perf_counter() - t0)
            r.last_out = oa
        return ws

    r1 = R(nc1)
    r2 = R(nc2)
    timed_calls(r1, 2)  # warm (load executables)
    timed_calls(r2, 2)

    walls1, walls2, marginals = [], [], []
    for _ in range(rounds):
        w1 = timed_calls(r1, iters_per_round)
        w2 = timed_calls(r2, iters_per_round)
        walls1 += w1
        walls2 += w2
        marginals.append((min(w2) - min(w1)) / (REPK - 1))
    marginals.sort()
    marginal = marginals[len(marginals) // 2]
    marginal = max(marginal, 2e-5)

    i_out = r1.out_names.index("out")
    npc = plan["npc"]
    out = np.asarray(r1.last_out[i_out]).reshape(
        n_cores * npc, F3).astype(np.float32)
    out2 = np.asarray(r2.last_out[i_out]).reshape(
        n_cores * npc, F3).astype(np.float32)
    assert np.allclose(out, out2, atol=1e-3, rtol=1e-2), \
        "repeat=2 executable disagrees with repeat=1"
    return out, marginal, walls1, walls2


def kernel(**inputs):
    x = np.asarray(inputs["x"], dtype=np.float32)
    N, DIN = x.shape
    F1 = inputs["W1"].shape[1]
    F2 = inputs["W2"].shape[1]
    F3 = inputs["W3"].shape[1]
    edge_index = np.asarray(inputs["edge_index"])

    key = (N, DIN, F1, F2, F3, hash(edge_index.tobytes()))
    if key not in _CACHE:
        plan = make_plan(edge_index, N, sort_src=V2)
        nc = (build_kernel2 if V2 else build_kernel)(plan, DIN, F1, F2, F3)
        _CACHE[key] = (plan, nc)
    plan, nc = _CACHE[key]

    in_maps = (make_in_maps2 if V2 else make_in_maps)(
        plan, inputs, DIN, F1, F2, F3)
    res = run_bass_kernel_spmd(nc, in_maps, core_ids=list(range(plan["n_cores"])))
    out = np.concatenate([res.results[r]["out"] for r in range(plan["n_cores"])],
                         axis=0)
    return out.astype(np.float32)


if __name__ == "__main__":
    import reference

    inputs = {k: np.asarray(v) for k, v in reference.setup_inputs().items()}
    out = kernel(**inputs)
    exp = np.asarray(reference.reference(**inputs))
    err = np.abs(out - exp).max() / (np.abs(exp).max() + 1e-30)
    print("Relative error:", err)



# revision 34
# speedup vs baseline: 1.0263x; 1.0263x over previous
"""Trainium2 Bass kernel for a 3-layer GCN (DeepGRL) on 8 NeuronCores.

Sharding: nodes are sharded contiguously across the 8 cores; edges are owned
by the core that owns their destination node; weights/BN params replicated.

The shipped kernel (build_kernel2) uses aggregate-first ordering,
out = (A_hat @ z) @ W, which needs one less table AllGather than the
multiply-first form and no layer-1 collective at all:

  - Every core holds a full node-major DRAM table of y = dinv * z rows
    (bf16).  Layer 1's table is just dinv * x, staged replicated as an
    ExternalInput - no collective.  Layer 3 produces only local output
    rows - no collective.  Only the two BN layers AllGather their tables
    (in two halves so gathers on half A overlap the AllGather of half B).
  - Aggregation per 128-dst-node block: edge source rows are fetched with
    the SWDGE dma_gather instruction (128 rows per chunk land on 128
    partitions); a one-hot segment matrix S (vector-engine is_equal of
    dst-local ids vs an iota row) is the matmul rhs, so the PSUM
    accumulation emits aggT feature-major.  The self-loop term enters the
    same accumulation as one identity matmul of the local y block.
  - aggT feeds the dense matmul directly as lhsT (no transposes anywhere):
    u = aggT^T @ W_eff -> node-major, scaled per-partition by dinv_dst.
  - BatchNorm (training-mode, biased var) batch stats are ones-vector
    matmuls, AllReduce'd across cores.  Using relu(A*u+B) = A*relu(u+C),
    C = B/A (A = g*rsqrt(var+eps) > 0), the per-feature scale A is folded
    exactly into the next layer's weight rows (f32 activation scale) and
    only the bias C is applied node-major (one DVE add against a [128,128]
    broadcast tile built with two small PE matmuls), then ReLU + dinv
    pre-scale in one activation.

dma_gather indices are int16, so the gathered table is addressed in two
halves (rows < HALF and rows >= HALF); every dst block's edge list is split
into a "lo" and a "hi" sublist, each padded to a multiple of 128.

Measured (marginal repeat=8 vs repeat=1 device time per network pass):
1237 us for the previous multiply-first kernel -> ~930-990 us for this one;
collectives are ~70 us of that, gather+segment-matmul pipeline ~790 us.
The gather pipeline is PER-DESCRIPTOR bound, not byte bound: a timing-only
variant moving the same bytes with half the descriptors (512B each) ran
~320 us/pass faster (~2.4 ns marginal cost per 256B descriptor).  Random
edges need one descriptor per edge, so this is the floor for this layout;
edge lists are source-sorted (sort_src=True) for slightly better HBM
locality.  dma_gather(single_packet=True) hangs the device - never use it.
"""

import math
from contextlib import ExitStack

import numpy as np

import concourse.bacc as bacc
import concourse.bass as bass
import concourse.mybir as mybir
import concourse.tile as tile
from concourse import library_config
from concourse.bass_utils import run_bass_kernel_spmd

P = 128
F32 = mybir.dt.float32
BF16 = mybir.dt.bfloat16
I16 = mybir.dt.int16
AF = mybir.ActivationFunctionType
ALU = mybir.AluOpType


# ----------------------------------------------------------------------------
# Host-side graph preprocessing
# ----------------------------------------------------------------------------
def make_plan(edge_index, N, n_cores=8, nblkA=25, gb=4, sort_src=False):
    """Partition edges by destination core, build per-core gather index /
    segment-id arrays (compile-time constants of the kernel).

    The u table is split into two AllGather'd halves by per-core block
    range: blocks [0, nblkA) -> table A, blocks [nblkA, nblk) -> table B.
    Gathers from table A can start as soon as AllGather-A lands, while
    AllGather-B is still in flight.  (lo == A, hi == B throughout.)"""
    src = np.asarray(edge_index[0], dtype=np.int64)
    dst = np.asarray(edge_index[1], dtype=np.int64)
    E = src.shape[0]

    indeg = np.bincount(dst, minlength=N).astype(np.float64)
    deg = indeg + 1.0  # self loop
    dinv = (1.0 / np.sqrt(deg)).astype(np.float32)

    npc = N // n_cores
    assert npc * n_cores == N
    nblk = math.ceil(npc / P)
    npc_pad = nblk * P
    nblkB = nblk - nblkA
    rA = nblkA * P  # per-core rows in table A
    rB = nblkB * P
    assert n_cores * rA < 32768 and n_cores * rB < 32768  # int16 gather idx

    dst_core = dst // npc
    dloc = dst - dst_core * npc

    # source row in the two-table layout
    src_core = src // npc
    src_loc = src - src_core * npc

    # bucket edges: per (core, block) -> lo (table A) / hi (table B) list
    lo_lists = [[[] for _ in range(nblk)] for _ in range(n_cores)]
    hi_lists = [[[] for _ in range(nblk)] for _ in range(n_cores)]
    lo_dl = [[[] for _ in range(nblk)] for _ in range(n_cores)]
    hi_dl = [[[] for _ in range(nblk)] for _ in range(n_cores)]
    order = np.argsort(dst, kind="stable")
    for e in order:
        r = dst_core[e]
        b = dloc[e] // P
        d_in_blk = dloc[e] - b * P
        sl = src_loc[e]
        if sl < rA:
            lo_lists[r][b].append(src_core[e] * rA + sl)
            lo_dl[r][b].append(d_in_blk)
        else:
            hi_lists[r][b].append(src_core[e] * rB + (sl - rA))
            hi_dl[r][b].append(d_in_blk)

    if sort_src:
        # ascending table addresses within each block's chunk sequence ->
        # better HBM row-buffer / bank locality for the gather descriptors
        for r in range(n_cores):
            for b in range(nblk):
                for ids, dls in ((lo_lists[r][b], lo_dl[r][b]),
                                 (hi_lists[r][b], hi_dl[r][b])):
                    if ids:
                        pairs = sorted(zip(ids, dls))
                        ids[:] = [p[0] for p in pairs]
                        dls[:] = [p[1] for p in pairs]

    c_lo = max(
        1,
        max(
            math.ceil(len(lo_lists[r][b]) / P)
            for r in range(n_cores)
            for b in range(nblk)
        ),
    )
    n_hi = max(
        len(hi_lists[r][b]) for r in range(n_cores) for b in range(nblk)
    )
    c_hi = math.ceil(n_hi / P)  # may be 0
    cpb = c_lo + c_hi  # chunks per block

    # gather groups of up to `gb` blocks
    groups = []
    b0 = 0
    while b0 < nblk:
        g = min(gb, nblk - b0)
        groups.append((b0, g))
        b0 += g

    def wrap_idx(ids):
        """int16 wrap layout: idx i -> [i % 16, i // 16], replicated to 128
        partitions (8 groups of 16)."""
        ids = np.asarray(ids, dtype=np.int16)
        L = ids.shape[0]
        assert L % 16 == 0
        w = ids.reshape(L // 16, 16).T  # [16, L/16]
        return np.tile(w, (8, 1))  # [128, L/16]

    idx_lo = np.zeros((n_cores, P, nblk * c_lo * 8), dtype=np.int16)
    idx_hi = np.zeros((n_cores, P, max(1, nblk * c_hi * 8)), dtype=np.int16)
    dl_arr = np.full((n_cores, P, nblk * cpb), 300.0, dtype=np.float32)

    for r in range(n_cores):
        lo_col = 0
        hi_col = 0
        for b0, g in groups:
            lo_ids = []
            hi_ids = []
            for b in range(b0, b0 + g):
                ll = lo_lists[r][b]
                ll = ll + [0] * (c_lo * P - len(ll))
                lo_ids.extend(ll)
                hl = hi_lists[r][b]
                hl = hl + [0] * (c_hi * P - len(hl))
                hi_ids.extend(hl)
                # dst-local ids, chunk-major (lo chunks then hi chunks)
                dl_pad_lo = lo_dl[r][b] + [300] * (c_lo * P - len(lo_dl[r][b]))
                dl_pad_hi = hi_dl[r][b] + [300] * (c_hi * P - len(hi_dl[r][b]))
                dl_all = dl_pad_lo + dl_pad_hi
                for c in range(cpb):
                    dl_arr[r, :, b * cpb + c] = dl_all[c * P : (c + 1) * P]
            w = wrap_idx(lo_ids)
            idx_lo[r][:, lo_col : lo_col + w.shape[1]] = w
            lo_col += w.shape[1]
            if c_hi > 0:
                w = wrap_idx(hi_ids)
                idx_hi[r][:, hi_col : hi_col + w.shape[1]] = w
                hi_col += w.shape[1]

    dinv_own = np.zeros((n_cores, P, nblk), dtype=np.float32)
    for r in range(n_cores):
        own = dinv[r * npc : (r + 1) * npc]
        own = np.pad(own, (0, npc_pad - npc))
        dinv_own[r] = own.reshape(nblk, P).T

    return dict(
        n_cores=n_cores,
        N=N,
        E=E,
        nblkA=nblkA,
        rA=rA,
        rB=rB,
        npc=npc,
        nblk=nblk,
        npc_pad=npc_pad,
        c_lo=c_lo,
        c_hi=c_hi,
        groups=groups,
        idx_lo=idx_lo,
        idx_hi=idx_hi,
        dl=dl_arr,
        dinv_own=dinv_own,
    )


# ----------------------------------------------------------------------------
# Kernel builder (same BIR for all cores; per-core data via input tensors)
# ----------------------------------------------------------------------------
def build_kernel(plan, DIN, F1, F2, F3, skip=(), repeat=1, gbufs=3,
                 sbufs=4, tbufs=3):
    n_cores = plan["n_cores"]
    N = plan["N"]
    nblkA = plan["nblkA"]
    rA = plan["rA"]
    rB = plan["rB"]
    nblk = plan["nblk"]
    npc = plan["npc"]
    npc_pad = plan["npc_pad"]
    c_lo = plan["c_lo"]
    c_hi = plan["c_hi"]
    cpb = c_lo + c_hi
    groups = plan["groups"]
    rtA = n_cores * rA
    rtB = n_cores * rB
    rg = [list(range(n_cores))]

    nc = bacc.Bacc("TRN2", target_bir_lowering=False, debug=False,
                   num_devices=n_cores, num_swdge_queues=4)
    import itertools
    _gq = itertools.count()

    # ---- I/O ----
    aT0 = nc.dram_tensor("aT0", [P, npc_pad], BF16, kind="ExternalInput")
    W1 = nc.dram_tensor("W1", [P, F1], BF16, kind="ExternalInput")
    W2 = nc.dram_tensor("W2", [P, F2], BF16, kind="ExternalInput")
    W3 = nc.dram_tensor("W3", [P, F3], BF16, kind="ExternalInput")
    g1 = nc.dram_tensor("g1", [P, 1], F32, kind="ExternalInput")
    be1 = nc.dram_tensor("be1", [P, 1], F32, kind="ExternalInput")
    g2 = nc.dram_tensor("g2", [P, 1], F32, kind="ExternalInput")
    be2 = nc.dram_tensor("be2", [P, 1], F32, kind="ExternalInput")
    b3b = nc.dram_tensor("b3b", [P, F3], F32, kind="ExternalInput")
    iota_in = nc.dram_tensor("iota", [P, P], BF16, kind="ExternalInput")
    ident_in = nc.dram_tensor("ident", [P, P], BF16, kind="ExternalInput")
    idx_lo_in = nc.dram_tensor("idx_lo", list(plan["idx_lo"].shape[1:]), I16,
                               kind="ExternalInput")
    idx_hi_in = nc.dram_tensor("idx_hi", list(plan["idx_hi"].shape[1:]), I16,
                               kind="ExternalInput")
    dl_in = nc.dram_tensor("dl", [P, nblk * cpb], BF16, kind="ExternalInput")
    dinv_in = nc.dram_tensor("dinv_own", [P, nblk], F32, kind="ExternalInput")
    out_t = nc.dram_tensor("out", [npc, F3], F32, kind="ExternalOutput")

    with tile.TileContext(nc) as tc, ExitStack() as ctx:
        nc.gpsimd.load_library(library_config.mlp)

        sb = ctx.enter_context(tc.tile_pool(name="sb", bufs=1))
        # persistent sbuf state
        aT_a = sb.tile([P, npc_pad], BF16, tag="aT_a")
        aT_b = sb.tile([P, npc_pad], BF16, tag="aT_b")
        u_own = sb.tile([P, nblk, max(F1, F2)], BF16, tag="u_own")
        u_own3 = sb.tile([P, nblk, P], BF16, tag="u_own3")
        z_own = sb.tile([P, nblk, max(F1, F2)], BF16, tag="z_own")
        w_sb = sb.tile([P, F1 + F2 + F3], BF16, tag="w_sb")
        iota_t = sb.tile([P, P], BF16, tag="iota_t")
        ident_t = sb.tile([P, P], BF16, tag="ident_t")
        ones_t = sb.tile([P, 1], BF16, tag="ones_t")
        dinv_t = sb.tile([P, nblk], F32, tag="dinv_t")
        dl_t = sb.tile([P, nblk * cpb], BF16, tag="dl_t")
        ilo_t = sb.tile(list(plan["idx_lo"].shape[1:]), I16, tag="ilo_t")
        ihi_t = sb.tile(list(plan["idx_hi"].shape[1:]), I16, tag="ihi_t")
        bnp_t = sb.tile([P, 4], F32, tag="bnp_t")  # g1 be1 g2 be2
        b3_t = sb.tile([P, F3], F32, tag="b3_t")

        nc.sync.dma_start(w_sb[:, 0:F1], W1[:])
        nc.sync.dma_start(w_sb[:, F1:F1 + F2], W2[:])
        nc.sync.dma_start(w_sb[:, F1 + F2:], W3[:])
        nc.sync.dma_start(iota_t[:], iota_in[:])
        nc.sync.dma_start(ident_t[:], ident_in[:])
        nc.sync.dma_start(dinv_t[:], dinv_in[:])
        nc.sync.dma_start(dl_t[:], dl_in[:])
        nc.sync.dma_start(ilo_t[:], idx_lo_in[:])
        if c_hi > 0:
            nc.sync.dma_start(ihi_t[:], idx_hi_in[:])
        nc.sync.dma_start(bnp_t[:, 0:1], g1[:])
        nc.sync.dma_start(bnp_t[:, 1:2], be1[:])
        nc.sync.dma_start(bnp_t[:, 2:3], g2[:])
        nc.sync.dma_start(bnp_t[:, 3:4], be2[:])
        nc.sync.dma_start(b3_t[:], b3b[:])
        nc.sync.dma_start(aT_a[:], aT0[:])
        nc.gpsimd.memset(ones_t[:], 1.0)
        nc.gpsimd.memset(u_own3[:, :, F3:], 0.0)

        # DRAM scratch
        dram = ctx.enter_context(tc.tile_pool(name="dram", bufs=1,
                                              space="DRAM"))
        u1A = dram.tile([rA, F1], BF16, tag="u1A")
        u1B = dram.tile([rB, F1], BF16, tag="u1B")
        u2A = dram.tile([rA, F2], BF16, tag="u2A")
        u2B = dram.tile([rB, F2], BF16, tag="u2B")
        u3A = dram.tile([rA, P], BF16, tag="u3A")
        u3B = dram.tile([rB, P], BF16, tag="u3B")
        st_in1 = dram.tile([P, 2], F32, tag="st_in1")
        st_in2 = dram.tile([P, 2], F32, tag="st_in2")

        # working pools
        psum_mm = ctx.enter_context(
            tc.tile_pool(name="psum_mm", bufs=2, space="PSUM"))
        psum_agg = ctx.enter_context(
            tc.tile_pool(name="psum_agg", bufs=2, space="PSUM"))
        psum_st = ctx.enter_context(
            tc.tile_pool(name="psum_st", bufs=2, space="PSUM"))
        spool = ctx.enter_context(tc.tile_pool(name="spool", bufs=sbufs))
        gpool = ctx.enter_context(tc.tile_pool(name="gpool", bufs=gbufs))
        tpool = ctx.enter_context(tc.tile_pool(name="tpool", bufs=tbufs))

        gbmax = max(g for _, g in groups)

        def layer(l, aT_in, aT_out, F_in, F_out, w_off, udA, udB, ufA, ufB,
                  is_last, g_col=None, be_col=None, st_in=None, st_out=None):
            # ---------------- Phase A: dense matmul + u table ----------
            uo = u_own3 if is_last else u_own
            wtab = P if is_last else F_out

            def emit_half(ud, uf, b0, nb):
                nc.sync.dma_start(
                    ud[:].rearrange("(b p) f -> p b f", p=P),
                    uo[:, b0:b0 + nb, :wtab],
                )
                if n_cores > 1 and "coll" not in skip:
                    nc.gpsimd.collective_compute(
                        "AllGather", ALU.bypass, replica_groups=rg,
                        ins=[ud[:].opt()], outs=[uf[:].opt()],
                    )
                else:
                    nc.sync.dma_start(uf[0:nb * P, :], ud[:])

            for b in range(nblk):
                h_ps = psum_mm.tile([P, F_out], F32, tag="mm")
                nc.tensor.matmul(
                    h_ps[:],
                    lhsT=aT_in[:, b * P:(b + 1) * P],
                    rhs=w_sb[:, w_off:w_off + F_out],
                    start=True, stop=True,
                )
                nc.scalar.activation(uo[:, b, :F_out], h_ps[:], AF.Copy,
                                     scale=dinv_t[:, b:b + 1])
                if b == nblkA - 1:
                    emit_half(udA, ufA, 0, nblkA)
            emit_half(udB, ufB, nblkA, nblk - nblkA)

            # ---------------- Phase B: gather + segment matmul ---------
            if not is_last:
                st_s = psum_st.tile([P, 1], F32, tag="st_s")
                st_q = psum_st.tile([P, 1], F32, tag="st_q")
            lo_col = 0
            hi_col = 0
            for b0, g in groups:
                n_lo = g * c_lo * P
                lo_t = gpool.tile([P, gbmax * c_lo, P], BF16, tag="lo")
                if "noload" in skip:
                    pass
                elif "seqload" in skip:
                    nc.gpsimd.dma_start(
                        lo_t[:, :g * c_lo, :],
                        ufA[0:n_lo, :].rearrange(
                            "(c p) f -> p c f", p=P),
                    )
                else:
                    nc.gpsimd.dma_gather(
                        lo_t[:, :g * c_lo, :], ufA[:],
                        ilo_t[:, lo_col:lo_col + n_lo // 16],
                        n_lo, n_lo, P, single_packet=False,
                        queue_num=next(_gq) % 4,
                    )
                lo_col += n_lo // 16
                if c_hi > 0:
                    n_hi = g * c_hi * P
                    hi_t = gpool.tile([P, gbmax * c_hi, P], BF16, tag="hi")
                    if "noload" in skip:
                        pass
                    elif "seqload" in skip:
                        nc.gpsimd.dma_start(
                            hi_t[:, :g * c_hi, :],
                            ufB[0:n_hi, :].rearrange(
                                "(c p) f -> p c f", p=P),
                        )
                    else:
                        nc.gpsimd.dma_gather(
                            hi_t[:, :g * c_hi, :],
                            ufB[:],
                            ihi_t[:, hi_col:hi_col + n_hi // 16],
                            n_hi, n_hi, P, single_packet=False,
                            queue_num=next(_gq) % 4,
                        )
                    hi_col += n_hi // 16
                for bb in range(g):
                    b = b0 + bb
                    agg = psum_agg.tile([P, F_out], F32, tag="agg")
                    if "seg" in skip:
                        nc.vector.memset(agg[:], 0.0)
                    else:
                        s_w = spool.tile([P, cpb, P], BF16, tag="s")
                        nc.vector.tensor_tensor(
                            out=s_w[:],
                            in0=iota_t[:, None, :].to_broadcast([P, cpb, P]),
                            in1=dl_t[:, b * cpb:(b + 1) * cpb].to_broadcast(
                                [P, cpb, P]),
                            op=ALU.is_equal,
                        )
                        for c in range(cpb):
                            if c < c_lo:
                                rhs = lo_t[:, bb * c_lo + c, :F_out]
                            else:
                                rhs = hi_t[:, bb * c_hi + (c - c_lo), :F_out]
                            nc.tensor.matmul(
                                agg[:], lhsT=s_w[:, c, :], rhs=rhs,
                                start=(c == 0), stop=(c == cpb - 1),
                            )
                    # epilogue: z = dinv * (agg + u_own)
                    uo = u_own3 if is_last else u_own
                    t_t = tpool.tile([P, F_out], F32, tag="t")
                    nc.vector.tensor_tensor(
                        out=t_t[:], in0=agg[:], in1=uo[:, b, :F_out],
                        op=ALU.add,
                    )
                    if is_last:
                        z3 = tpool.tile([P, F_out], F32, tag="z3")
                        nc.scalar.activation(z3[:], t_t[:], AF.Copy,
                                             scale=dinv_t[:, b:b + 1])
                        o_t = tpool.tile([P, F_out], F32, tag="o")
                        nc.vector.tensor_tensor(out=o_t[:], in0=z3[:],
                                                in1=b3_t[:], op=ALU.add)
                        hi_row = min(npc, (b + 1) * P) - b * P
                        nc.sync.dma_start(out_t[b * P:b * P + hi_row, :],
                                          o_t[:hi_row, :])
                    else:
                        nc.scalar.activation(z_own[:, b, :F_out], t_t[:],
                                             AF.Copy,
                                             scale=dinv_t[:, b:b + 1])
                        z2 = tpool.tile([P, F_out], BF16, tag="z2")
                        nc.scalar.activation(z2[:], z_own[:, b, :F_out],
                                             AF.Square)
                        nc.tensor.matmul(st_s[:], lhsT=z_own[:, b, :F_out],
                                         rhs=ones_t[:],
                                         start=(b == 0), stop=(b == nblk - 1),
                                         skip_group_check=True)
                        nc.tensor.matmul(st_q[:], lhsT=z2[:], rhs=ones_t[:],
                                         start=(b == 0), stop=(b == nblk - 1),
                                         skip_group_check=True)
            if is_last:
                return

            # ---------------- Phase C: BN stats allreduce + coeffs -----
            st_sb = tpool.tile([P, 2], F32, tag="stsb")
            nc.vector.tensor_copy(st_sb[:, 0:1], st_s[:])
            nc.vector.tensor_copy(st_sb[:, 1:2], st_q[:])
            nc.sync.dma_start(st_in[:], st_sb[:])
            if n_cores > 1 and "coll" not in skip:
                nc.gpsimd.collective_compute(
                    "AllReduce", ALU.add, replica_groups=rg,
                    ins=[st_in[:].opt()], outs=[st_out[:].opt()],
                )
            else:
                nc.sync.dma_start(st_out[:], st_in[:])
            st_g = tpool.tile([P, 2], F32, tag="stg")
            nc.sync.dma_start(st_g[:], st_out[:])
            m_t = tpool.tile([P, 1], F32, tag="m")
            nc.scalar.activation(m_t[:], st_g[:, 0:1], AF.Copy, scale=1.0 / N)
            q_t = tpool.tile([P, 1], F32, tag="q")
            nc.scalar.activation(q_t[:], st_g[:, 1:2], AF.Copy, scale=1.0 / N)
            m2_t = tpool.tile([P, 1], F32, tag="m2")
            nc.scalar.activation(m2_t[:], m_t[:], AF.Square)
            v_t = tpool.tile([P, 1], F32, tag="v")
            nc.vector.tensor_tensor(out=v_t[:], in0=q_t[:], in1=m2_t[:],
                                    op=ALU.subtract)
            ve_t = tpool.tile([P, 1], F32, tag="ve")
            nc.vector.tensor_scalar(out=ve_t[:], in0=v_t[:], scalar1=1e-5,
                                    scalar2=None, op0=ALU.add)
            sd_t = tpool.tile([P, 1], F32, tag="sd")
            nc.scalar.activation(sd_t[:], ve_t[:], AF.Sqrt)
            inv_t = tpool.tile([P, 1], F32, tag="inv")
            nc.vector.reciprocal(inv_t[:], sd_t[:])
            a_t = tpool.tile([P, 1], F32, tag="A")
            nc.vector.tensor_tensor(out=a_t[:], in0=bnp_t[:, g_col:g_col + 1],
                                    in1=inv_t[:], op=ALU.mult)
            ma_t = tpool.tile([P, 1], F32, tag="mA")
            nc.vector.tensor_tensor(out=ma_t[:], in0=m_t[:], in1=a_t[:],
                                    op=ALU.mult)
            bb_t = tpool.tile([P, 1], F32, tag="B")
            nc.vector.tensor_tensor(out=bb_t[:],
                                    in0=bnp_t[:, be_col:be_col + 1],
                                    in1=ma_t[:], op=ALU.subtract)

            # ---------------- Phase D: transpose + BN apply + relu -----
            for b in range(nblk):
                zT = psum_mm.tile([P, P], BF16, tag="mm")
                nc.tensor.transpose(zT[:], z_own[:, b, :F_out], ident_t[:])
                nc.scalar.activation(aT_out[:, b * P:(b + 1) * P], zT[:],
                                     AF.Relu, bias=bb_t[:], scale=a_t[:])

        for _rep in range(repeat):
            uf1A = dram.tile([rtA, F1], BF16, tag=f"uf1A_{_rep}",
                             addr_space="Shared")
            uf1B = dram.tile([rtB, F1], BF16, tag=f"uf1B_{_rep}",
                             addr_space="Shared")
            uf2A = dram.tile([rtA, F2], BF16, tag=f"uf2A_{_rep}",
                             addr_space="Shared")
            uf2B = dram.tile([rtB, F2], BF16, tag=f"uf2B_{_rep}",
                             addr_space="Shared")
            uf3A = dram.tile([rtA, P], BF16, tag=f"uf3A_{_rep}",
                             addr_space="Shared")
            uf3B = dram.tile([rtB, P], BF16, tag=f"uf3B_{_rep}",
                             addr_space="Shared")
            st_out1 = dram.tile([P, 2], F32, tag=f"st_out1_{_rep}",
                                addr_space="Shared")
            st_out2 = dram.tile([P, 2], F32, tag=f"st_out2_{_rep}",
                                addr_space="Shared")
            if _rep > 0:
                nc.sync.dma_start(aT_a[:], aT0[:])
            layer(1, aT_a, aT_b, DIN, F1, 0, u1A, u1B, uf1A, uf1B, False,
                  0, 1, st_in1, st_out1)
            layer(2, aT_b, aT_a, F1, F2, F1, u2A, u2B, uf2A, uf2B, False,
                  2, 3, st_in2, st_out2)
            layer(3, aT_a, None, F2, F3, F1 + F2, u3A, u3B, uf3A, uf3B,
                  True)

    nc.compile()
    return nc


# ----------------------------------------------------------------------------
# Kernel v2: aggregate-first ordering  out = (A_hat @ z) @ W
#   - the gathered table holds y = dinv * z (node-major rows, bf16); layer 1
#     gathers straight from the replicated input tables (no AllGather);
#     layer 3 needs no table at all -> only 2 table AllGather pairs total.
#   - the self-loop term enters the PSUM accumulation as one identity matmul
#     of the local y block (no extra gather/transpose/add).
#   - segment matmul emits aggT feature-major, which feeds the dense matmul
#     as lhsT directly; BN+ReLU applied node-major on DVE with exact f32
#     per-feature rows (block-transpose + partition_broadcast).
# ----------------------------------------------------------------------------
def build_kernel2(plan, DIN, F1, F2, F3, skip=(), repeat=1, gbufs=4,
                  sbufs=4, tbufs=6, abufs=2, mbufs=2, dbg=(),
                  single_packet=False, nqueues=4):
    n_cores = plan["n_cores"]
    N = plan["N"]
    nblkA = plan["nblkA"]
    rA = plan["rA"]
    rB = plan["rB"]
    nblk = plan["nblk"]
    npc = plan["npc"]
    c_lo = plan["c_lo"]
    c_hi = plan["c_hi"]
    cpb = c_lo + c_hi
    groups = plan["groups"]
    rtA = n_cores * rA
    rtB = n_cores * rB
    rg = [list(range(n_cores))]
    assert DIN == 128 and F1 == 128 and F2 == 128

    nc = bacc.Bacc("TRN2", target_bir_lowering=False, debug=False,
                   num_devices=n_cores, num_swdge_queues=nqueues)
    import itertools
    _gq = itertools.count()

    # ---- I/O ----
    tA0 = nc.dram_tensor("tA0", [rtA, DIN], BF16, kind="ExternalInput")
    tB0 = nc.dram_tensor("tB0", [rtB, DIN], BF16, kind="ExternalInput")
    yown0 = nc.dram_tensor("yown0", [P, nblk * DIN], BF16,
                           kind="ExternalInput")
    W1 = nc.dram_tensor("W1", [P, F1], BF16, kind="ExternalInput")
    W2 = nc.dram_tensor("W2", [P, F2], BF16, kind="ExternalInput")
    W3 = nc.dram_tensor("W3", [P, F3], BF16, kind="ExternalInput")
    g1 = nc.dram_tensor("g1", [P, 1], F32, kind="ExternalInput")
    be1 = nc.dram_tensor("be1", [P, 1], F32, kind="ExternalInput")
    g2 = nc.dram_tensor("g2", [P, 1], F32, kind="ExternalInput")
    be2 = nc.dram_tensor("be2", [P, 1], F32, kind="ExternalInput")
    b3b = nc.dram_tensor("b3b", [P, F3], F32, kind="ExternalInput")
    iota_in = nc.dram_tensor("iota", [P, P], BF16, kind="ExternalInput")
    ident_in = nc.dram_tensor("ident", [P, P], BF16, kind="ExternalInput")
    idx_lo_in = nc.dram_tensor("idx_lo", list(plan["idx_lo"].shape[1:]), I16,
                               kind="ExternalInput")
    idx_hi_in = nc.dram_tensor("idx_hi", list(plan["idx_hi"].shape[1:]), I16,
                               kind="ExternalInput")
    dl_in = nc.dram_tensor("dl", [P, nblk * cpb], BF16, kind="ExternalInput")
    dinv_in = nc.dram_tensor("dinv_own", [P, nblk], F32, kind="ExternalInput")
    out_t = nc.dram_tensor("out", [npc, F3], F32, kind="ExternalOutput")
    dbg_t = {}
    for d in dbg:
        if d in ("u1", "y1"):
            dbg_t[d] = nc.dram_tensor(f"dbg_{d}", [P, nblk * 128], BF16,
                                      kind="ExternalOutput")
        else:
            dbg_t[d] = nc.dram_tensor(f"dbg_{d}", [P, 128], F32,
                                      kind="ExternalOutput")

    with tile.TileContext(nc) as tc, ExitStack() as ctx:
        nc.gpsimd.load_library(library_config.mlp)

        sb = ctx.enter_context(tc.tile_pool(name="sb", bufs=1))
        w_sb = sb.tile([P, F1 + F2 + F3], BF16, tag="w_sb")
        iota_t = sb.tile([P, P], BF16, tag="iota_t")
        ident_t = sb.tile([P, P], BF16, tag="ident_t")
        ones_t = sb.tile([P, 1], BF16, tag="ones_t")
        ones_r = sb.tile([1, P], BF16, tag="ones_r")
        dinv_t = sb.tile([P, nblk], F32, tag="dinv_t")
        dl_t = sb.tile([P, nblk * cpb], BF16, tag="dl_t")
        ilo_t = sb.tile(list(plan["idx_lo"].shape[1:]), I16, tag="ilo_t")
        ihi_t = sb.tile(list(plan["idx_hi"].shape[1:]), I16, tag="ihi_t")
        bnp_t = sb.tile([P, 4], F32, tag="bnp_t")
        b3_t = sb.tile([P, F3], F32, tag="b3_t")
        y_a = sb.tile([P, nblk, 128], BF16, tag="y_a")
        y_b = sb.tile([P, nblk, 128], BF16, tag="y_b")
        u_keep = sb.tile([P, nblk, 128], BF16, tag="u_keep")

        nc.sync.dma_start(w_sb[:, 0:F1], W1[:])
        nc.sync.dma_start(w_sb[:, F1:F1 + F2], W2[:])
        nc.sync.dma_start(w_sb[:, F1 + F2:], W3[:])
        nc.sync.dma_start(iota_t[:], iota_in[:])
        nc.sync.dma_start(ident_t[:], ident_in[:])
        nc.sync.dma_start(dinv_t[:], dinv_in[:])
        nc.sync.dma_start(dl_t[:], dl_in[:])
        nc.sync.dma_start(ilo_t[:], idx_lo_in[:])
        if c_hi > 0:
            nc.sync.dma_start(ihi_t[:], idx_hi_in[:])
        nc.sync.dma_start(bnp_t[:, 0:1], g1[:])
        nc.sync.dma_start(bnp_t[:, 1:2], be1[:])
        nc.sync.dma_start(bnp_t[:, 2:3], g2[:])
        nc.sync.dma_start(bnp_t[:, 3:4], be2[:])
        nc.sync.dma_start(b3_t[:], b3b[:])
        nc.sync.dma_start(
            y_a[:], yown0[:].rearrange("p (b f) -> p b f", b=nblk))
        nc.gpsimd.memset(ones_t[:], 1.0)
        nc.gpsimd.memset(ones_r[:], 1.0)

        dram = ctx.enter_context(tc.tile_pool(name="dram", bufs=1,
                                              space="DRAM"))
        d1A = dram.tile([rA, F1], BF16, tag="d1A")
        d1B = dram.tile([rB, F1], BF16, tag="d1B")
        d2A = dram.tile([rA, F2], BF16, tag="d2A")
        d2B = dram.tile([rB, F2], BF16, tag="d2B")
        st_in1 = dram.tile([P, 2], F32, tag="st_in1")
        st_in2 = dram.tile([P, 2], F32, tag="st_in2")

        psum_agg = ctx.enter_context(
            tc.tile_pool(name="psum_agg", bufs=abufs, space="PSUM"))
        psum_mm = ctx.enter_context(
            tc.tile_pool(name="psum_mm", bufs=mbufs, space="PSUM"))
        psum_st = ctx.enter_context(
            tc.tile_pool(name="psum_st", bufs=2, space="PSUM"))
        spool = ctx.enter_context(tc.tile_pool(name="spool", bufs=sbufs))
        gpool = ctx.enter_context(tc.tile_pool(name="gpool", bufs=gbufs))
        tpool = ctx.enter_context(tc.tile_pool(name="tpool", bufs=tbufs))

        gbmax = max(g for _, g in groups)

        def emit_half(y_src, ud, uf, b0, nb, F_out):
            nc.sync.dma_start(
                ud[:].rearrange("(b p) f -> p b f", p=P),
                y_src[:, b0:b0 + nb, :F_out],
            )
            if n_cores > 1 and "coll" not in skip:
                nc.gpsimd.collective_compute(
                    "AllGather", ALU.bypass, replica_groups=rg,
                    ins=[ud[:].opt()], outs=[uf[:].opt()],
                )
            else:
                nc.sync.dma_start(uf[0:nb * P, :], ud[:])

        def layer2(l, y_cur, y_nxt, F_out, w_rhs, tfA, tfB, udA=None,
                   udB=None, tfA_out=None, tfB_out=None, g_col=None,
                   be_col=None, st_in=None, st_out=None, w_next=None):
            """One GCN layer, aggregate-first.  w_rhs: SBUF [128, F_out]
            effective weight (previous layer's BN scale pre-folded).
            w_next: SBUF slice of the NEXT layer's raw weight; returns its
            A-scaled copy for the next layer2 call."""
            last = (l == 3)
            if not last:
                st_s = psum_st.tile([P, 1], F32, tag="st_s", bufs=1)
                st_q = psum_st.tile([P, 1], F32, tag="st_q", bufs=1)
            d512 = "d512" in skip  # timing diagnostic: half descs, 512B
            cd = lambda n: (n + 1) // 2
            lo_col = 0
            hi_col = 0
            for b0, g in groups:
                n_lo = g * c_lo * P
                if d512:
                    lo_t = gpool.tile([P, cd(gbmax * c_lo), 2 * P], BF16,
                                      tag="lo")
                    nc.gpsimd.dma_gather(
                        lo_t[:, :cd(g * c_lo), :],
                        tfA[:].rearrange("(r two) f -> r (two f)", two=2),
                        ilo_t[:, lo_col:lo_col + (n_lo // 2) // 16],
                        n_lo // 2, n_lo // 2, 2 * P,
                        single_packet=single_packet,
                        queue_num=next(_gq) % nqueues,
                    )
                else:
                    lo_t = gpool.tile([P, gbmax * c_lo, P], BF16, tag="lo")
                    if "gsm" not in skip:
                        nc.gpsimd.dma_gather(
                            lo_t[:, :g * c_lo, :], tfA[:],
                            ilo_t[:, lo_col:lo_col + n_lo // 16],
                            n_lo, n_lo, P, single_packet=single_packet,
                            queue_num=next(_gq) % nqueues,
                        )
                lo_col += n_lo // 16
                if c_hi > 0:
                    n_hi = g * c_hi * P
                    if d512:
                        hi_t = gpool.tile([P, cd(gbmax * c_hi), 2 * P], BF16,
                                          tag="hi")
                        nc.gpsimd.dma_gather(
                            hi_t[:, :cd(g * c_hi), :],
                            tfB[:].rearrange("(r two) f -> r (two f)", two=2),
                            ihi_t[:, hi_col:hi_col + (n_hi // 2) // 16],
                            n_hi // 2, n_hi // 2, 2 * P,
                            single_packet=single_packet,
                            queue_num=next(_gq) % nqueues,
                        )
                    else:
                        hi_t = gpool.tile([P, gbmax * c_hi, P], BF16,
                                          tag="hi")
                        if "gsm" not in skip:
                            nc.gpsimd.dma_gather(
                                hi_t[:, :g * c_hi, :], tfB[:],
                                ihi_t[:, hi_col:hi_col + n_hi // 16],
                                n_hi, n_hi, P, single_packet=single_packet,
                                queue_num=next(_gq) % nqueues,
                            )
                    hi_col += n_hi // 16
                for bb in range(g):
                    b = b0 + bb
                    agg = psum_agg.tile([P, P], F32, tag="agg")
                    if "gsm" in skip:
                        nc.tensor.matmul(agg[:], lhsT=y_cur[:, b, :],
                                         rhs=ident_t[:], start=True,
                                         stop=True)
                    else:
                        s_w = spool.tile([P, cpb, P], BF16, tag="s")
                        nc.vector.tensor_tensor(
                            out=s_w[:],
                            in0=iota_t[:, None, :].to_broadcast([P, cpb, P]),
                            in1=dl_t[:, b * cpb:(b + 1) * cpb].to_broadcast(
                                [P, cpb, P]),
                            op=ALU.is_equal,
                        )
                        nc.tensor.matmul(agg[:], lhsT=y_cur[:, b, :],
                                         rhs=ident_t[:], start=True,
                                         stop=False)
                        for c in range(cpb):
                            if c < c_lo:
                                i, t = bb * c_lo + c, lo_t
                            else:
                                i, t = bb * c_hi + (c - c_lo), hi_t
                            if d512:
                                lhs = t[:, i // 2,
                                        (i % 2) * P:(i % 2 + 1) * P]
                            else:
                                lhs = t[:, i, :]
                            nc.tensor.matmul(agg[:], lhsT=lhs,
                                             rhs=s_w[:, c, :],
                                             start=False, stop=(c == cpb - 1))
                    agg_sb = tpool.tile([P, P], BF16, tag="aggsb")
                    nc.scalar.activation(agg_sb[:], agg[:], AF.Copy)
                    u_ps = psum_mm.tile([P, F_out], F32, tag="u")
                    nc.tensor.matmul(u_ps[:], lhsT=agg_sb[:], rhs=w_rhs,
                                     start=True, stop=True)
                    if last:
                        o_t = tpool.tile([P, F_out], F32, tag="o")
                        nc.scalar.activation(o_t[:], u_ps[:], AF.Copy,
                                             scale=dinv_t[:, b:b + 1])
                        o2 = tpool.tile([P, F_out], F32, tag="o2")
                        nc.vector.tensor_tensor(out=o2[:], in0=o_t[:],
                                                in1=b3_t[:], op=ALU.add)
                        hi_row = min(npc, (b + 1) * P) - b * P
                        nc.sync.dma_start(out_t[b * P:b * P + hi_row, :],
                                          o2[:hi_row, :])
                    else:
                        nc.scalar.activation(u_keep[:, b, :F_out], u_ps[:],
                                             AF.Copy, scale=dinv_t[:, b:b + 1])
                        u2 = tpool.tile([P, F_out], BF16, tag="u2")
                        nc.scalar.activation(u2[:], u_keep[:, b, :F_out],
                                             AF.Square)
                        nc.tensor.matmul(st_s[:], lhsT=u_keep[:, b, :F_out],
                                         rhs=ones_t[:], start=(b == 0),
                                         stop=(b == nblk - 1),
                                         skip_group_check=True)
                        nc.tensor.matmul(st_q[:], lhsT=u2[:], rhs=ones_t[:],
                                         start=(b == 0), stop=(b == nblk - 1),
                                         skip_group_check=True)
            if last:
                return

            if l == 1 and "u1" in dbg_t:
                nc.sync.dma_start(
                    dbg_t["u1"][:].rearrange("p (b f) -> p b f", b=nblk),
                    u_keep[:])

            # BN stats allreduce + coefficient columns
            st_sb = tpool.tile([P, 2], F32, tag="stsb")
            nc.vector.tensor_copy(st_sb[:, 0:1], st_s[:])
            nc.vector.tensor_copy(st_sb[:, 1:2], st_q[:])
            nc.sync.dma_start(st_in[:], st_sb[:])
            if n_cores > 1 and "coll" not in skip:
                nc.gpsimd.collective_compute(
                    "AllReduce", ALU.add, replica_groups=rg,
                    ins=[st_in[:].opt()], outs=[st_out[:].opt()],
                )
            else:
                nc.sync.dma_start(st_out[:], st_in[:])
            st_g = tpool.tile([P, 2], F32, tag="stg")
            nc.sync.dma_start(st_g[:], st_out[:])
            m_t = tpool.tile([P, 1], F32, tag="m")
            nc.scalar.activation(m_t[:], st_g[:, 0:1], AF.Copy, scale=1.0 / N)
            q_t = tpool.tile([P, 1], F32, tag="q")
            nc.scalar.activation(q_t[:], st_g[:, 1:2], AF.Copy, scale=1.0 / N)
            m2_t = tpool.tile([P, 1], F32, tag="m2")
            nc.scalar.activation(m2_t[:], m_t[:], AF.Square)
            v_t = tpool.tile([P, 1], F32, tag="v")
            nc.vector.tensor_tensor(out=v_t[:], in0=q_t[:], in1=m2_t[:],
                                    op=ALU.subtract)
            ve_t = tpool.tile([P, 1], F32, tag="ve")
            nc.vector.tensor_scalar(out=ve_t[:], in0=v_t[:], scalar1=1e-5,
                                    scalar2=None, op0=ALU.add)
            sd_t = tpool.tile([P, 1], F32, tag="sd")
            nc.scalar.activation(sd_t[:], ve_t[:], AF.Sqrt)
            inv_t = tpool.tile([P, 1], F32, tag="inv")
            nc.vector.reciprocal(inv_t[:], sd_t[:])
            a_t = tpool.tile([P, 1], F32, tag="A")
            nc.vector.tensor_tensor(out=a_t[:], in0=bnp_t[:, g_col:g_col + 1],
                                    in1=inv_t[:], op=ALU.mult)
            ma_t = tpool.tile([P, 1], F32, tag="mA")
            nc.vector.tensor_tensor(out=ma_t[:], in0=m_t[:], in1=a_t[:],
                                    op=ALU.mult)
            bb_t = tpool.tile([P, 1], F32, tag="B")
            nc.vector.tensor_tensor(out=bb_t[:],
                                    in0=bnp_t[:, be_col:be_col + 1],
                                    in1=ma_t[:], op=ALU.subtract)

            # BN rewrite for A>0:  z = A*u + B -> relu(z) = A * relu(u + C),
            # C = B/A.  The table stores y~ = dinv * relu(u + C) (per-feature
            # scale A folded into the NEXT layer's weight rows, exactly);
            # the bias C is broadcast to a full [128,128] tile via two PE
            # matmuls (column -> row -> rank-1 broadcast).
            rc_t = tpool.tile([P, 1], F32, tag="rc")
            nc.vector.reciprocal(rc_t[:], a_t[:])
            c_t = tpool.tile([P, 1], F32, tag="c")
            nc.vector.tensor_tensor(out=c_t[:], in0=bb_t[:], in1=rc_t[:],
                                    op=ALU.mult)
            c_bf = tpool.tile([P, 1], BF16, tag="cbf")
            nc.scalar.activation(c_bf[:], c_t[:], AF.Copy)
            crow_ps = psum_st.tile([1, P], F32, tag="crow", bufs=1)
            nc.tensor.matmul(crow_ps[:], lhsT=c_bf[:], rhs=ident_t[:],
                             start=True, stop=True)
            crow_sb = tpool.tile([1, P], BF16, tag="crowsb")
            nc.scalar.activation(crow_sb[:], crow_ps[:], AF.Copy)
            cf_ps = psum_st.tile([P, P], F32, tag="cf", bufs=1)
            nc.tensor.matmul(cf_ps[:], lhsT=ones_r[:], rhs=crow_sb[:],
                             start=True, stop=True)
            C_full = tpool.tile([P, P], F32, tag="Cfull")
            nc.scalar.activation(C_full[:], cf_ps[:], AF.Copy)
            # fold A into the next layer's weight rows (exact, f32 scale)
            w_eff = sb.tile([P, w_next.shape[-1]], BF16, tag=f"weff{l}")
            nc.scalar.activation(w_eff[:], w_next, AF.Copy, scale=a_t[:])

            if l == 1 and "st1" in dbg_t:
                stdbg = tpool.tile([P, 128], F32, tag="stdbg")
                nc.vector.memset(stdbg[:], 0.0)
                nc.vector.tensor_copy(stdbg[:, 0:2], st_g[:])
                nc.vector.tensor_copy(stdbg[:, 2:3], a_t[:])
                nc.vector.tensor_copy(stdbg[:, 3:4], bb_t[:])
                nc.sync.dma_start(dbg_t["st1"][:], stdbg[:])
            if l == 1 and "af1" in dbg_t:
                nc.sync.dma_start(dbg_t["af1"][:], C_full[:])

            # bias + relu + dinv pre-scale, node-major; emit halves
            for b in range(nblk):
                t2 = tpool.tile([P, F_out], F32, tag="t2")
                nc.vector.tensor_tensor(out=t2[:], in0=u_keep[:, b, :F_out],
                                        in1=C_full[:, :F_out], op=ALU.add)
                nc.scalar.activation(y_nxt[:, b, :F_out], t2[:], AF.Relu,
                                     scale=dinv_t[:, b:b + 1])
                if b == nblkA - 1:
                    emit_half(y_nxt, udA, tfA_out, 0, nblkA, F_out)
            emit_half(y_nxt, udB, tfB_out, nblkA, nblk - nblkA, F_out)
            if l == 1 and "y1" in dbg_t:
                nc.sync.dma_start(
                    dbg_t["y1"][:].rearrange("p (b f) -> p b f", b=nblk),
                    y_nxt[:])
            return w_eff

        for _rep in range(repeat):
            tf1A = dram.tile([rtA, F1], BF16, tag=f"tf1A_{_rep}",
                             addr_space="Shared")
            tf1B = dram.tile([rtB, F1], BF16, tag=f"tf1B_{_rep}",
                             addr_space="Shared")
            tf2A = dram.tile([rtA, F2], BF16, tag=f"tf2A_{_rep}",
                             addr_space="Shared")
            tf2B = dram.tile([rtB, F2], BF16, tag=f"tf2B_{_rep}",
                             addr_space="Shared")
            st_out1 = dram.tile([P, 2], F32, tag=f"st_out1_{_rep}",
                                addr_space="Shared")
            st_out2 = dram.tile([P, 2], F32, tag=f"st_out2_{_rep}",
                                addr_space="Shared")
            if _rep > 0:
                nc.sync.dma_start(
                    y_a[:], yown0[:].rearrange("p (b f) -> p b f", b=nblk))
            w2_eff = layer2(1, y_a, y_b, F1, w_sb[:, 0:F1], tA0, tB0,
                            d1A, d1B, tf1A, tf1B, 0, 1, st_in1, st_out1,
                            w_next=w_sb[:, F1:F1 + F2])
            w3_eff = layer2(2, y_b, y_a, F2, w2_eff[:], tf1A, tf1B,
                            d2A, d2B, tf2A, tf2B, 2, 3, st_in2, st_out2,
                            w_next=w_sb[:, F1 + F2:F1 + F2 + F3])
            layer2(3, y_a, None, F3, w3_eff[:], tf2A, tf2B)

    nc.compile()
    return nc


def make_in_maps2(plan, inputs, DIN, F1, F2, F3):
    n_cores = plan["n_cores"]
    npc = plan["npc"]
    npc_pad = plan["npc_pad"]
    nblk = plan["nblk"]
    rA = plan["rA"]
    rB = plan["rB"]
    N = plan["N"]
    x = np.asarray(inputs["x"], dtype=np.float32)
    edge_index = np.asarray(inputs["edge_index"])
    import ml_dtypes
    bf16 = ml_dtypes.bfloat16

    dst = np.asarray(edge_index[1], dtype=np.int64)
    deg = np.bincount(dst, minlength=N).astype(np.float64) + 1.0
    dinv = (1.0 / np.sqrt(deg)).astype(np.float32)
    y0 = (x * dinv[:, None]).astype(bf16)

    tA0 = np.zeros((n_cores * rA, DIN), bf16)
    tB0 = np.zeros((n_cores * rB, DIN), bf16)
    yown0 = np.zeros((n_cores, P, nblk * DIN), bf16)
    for c in range(n_cores):
        yc = np.zeros((npc_pad, DIN), bf16)
        yc[:npc] = y0[c * npc:(c + 1) * npc]
        tA0[c * rA:(c + 1) * rA] = yc[:rA]
        tB0[c * rB:(c + 1) * rB] = yc[rA:]
        yown0[c] = yc.reshape(nblk, P, DIN).transpose(1, 0, 2).reshape(
            P, nblk * DIN)

    iota = np.tile(np.arange(P)[None, :], (P, 1)).astype(bf16)
    ident = np.eye(P, dtype=bf16)
    b3b = np.tile(np.asarray(inputs["b3"], np.float32)[None, :], (P, 1))
    col = lambda v: np.asarray(v, np.float32).reshape(P, 1)
    in_maps = []
    for r in range(n_cores):
        in_maps.append({
            "tA0": tA0, "tB0": tB0, "yown0": yown0[r],
            "W1": np.asarray(inputs["W1"], np.float32).astype(bf16),
            "W2": np.asarray(inputs["W2"], np.float32).astype(bf16),
            "W3": np.asarray(inputs["W3"], np.float32).astype(bf16),
            "g1": col(inputs["g1"]), "be1": col(inputs["be1"]),
            "g2": col(inputs["g2"]), "be2": col(inputs["be2"]),
            "b3b": b3b, "iota": iota, "ident": ident,
            "idx_lo": plan["idx_lo"][r], "idx_hi": plan["idx_hi"][r],
            "dl": plan["dl"][r].astype(bf16),
            "dinv_own": plan["dinv_own"][r],
        })
    return in_maps


# ----------------------------------------------------------------------------
# Host entry point
# ----------------------------------------------------------------------------
def make_in_maps(plan, inputs, DIN, F1, F2, F3):
    n_cores = plan["n_cores"]
    npc = plan["npc"]
    npc_pad = plan["npc_pad"]
    x = np.asarray(inputs["x"], dtype=np.float32)
    import ml_dtypes
    bf16 = ml_dtypes.bfloat16
    iota = np.tile(np.arange(P)[None, :], (P, 1)).astype(bf16)
    ident = np.eye(P, dtype=bf16)
    b3b = np.tile(np.asarray(inputs["b3"], np.float32)[None, :], (P, 1))
    col = lambda v: np.asarray(v, np.float32).reshape(P, 1)
    in_maps = []
    for r in range(n_cores):
        xr = x[r * npc:(r + 1) * npc]
        aT0 = np.zeros((P, npc_pad), bf16)
        aT0[:, :npc] = xr.T.astype(bf16)
        in_maps.append({
            "aT0": aT0,
            "W1": np.asarray(inputs["W1"], np.float32).astype(bf16),
            "W2": np.asarray(inputs["W2"], np.float32).astype(bf16),
            "W3": np.asarray(inputs["W3"], np.float32).astype(bf16),
            "g1": col(inputs["g1"]), "be1": col(inputs["be1"]),
            "g2": col(inputs["g2"]), "be2": col(inputs["be2"]),
            "b3b": b3b, "iota": iota, "ident": ident,
            "idx_lo": plan["idx_lo"][r], "idx_hi": plan["idx_hi"][r],
            "dl": plan["dl"][r].astype(ml_dtypes.bfloat16), "dinv_own": plan["dinv_own"][r],
        })
    return in_maps


_CACHE = {}


def _sharded_runner(nc, in_maps):
    """Build a single jit/shard_map executable for `nc` (same lowering path
    run_bass_kernel_spmd uses under axon) and return
    (call(dev_in, dev_zeros) -> out_arrs, stage() -> (dev_in, dev_zeros),
    out_names, out_avals)."""
    import jax
    from jax.sharding import Mesh, PartitionSpec, NamedSharding
    from jax.experimental.shard_map import shard_map
    from concourse.bass2jax import (
        _bass_exec_p, install_neuronx_cc_hook, partition_id_tensor)

    install_neuronx_cc_hook()
    n_cores = len(in_maps)
    partition_name = (
        nc.partition_id_tensor.name if nc.partition_id_tensor else None)
    in_names, out_names, out_avals, zero_outs = [], [], [], []
    for alloc in nc.m.functions[0].allocations:
        if not isinstance(alloc, mybir.MemoryLocationSet):
            continue
        name = alloc.memorylocations[0].name
        if alloc.kind == "ExternalInput":
            if name != partition_name:
                in_names.append(name)
        elif alloc.kind == "ExternalOutput":
            out_names.append(name)
            shape = tuple(alloc.tensor_shape)
            dtype = mybir.dt.np(alloc.dtype)
            out_avals.append(jax.core.ShapedArray(shape, dtype))
            zero_outs.append(np.zeros(shape, dtype))
    n_params = len(in_names)
    n_outs = len(out_avals)
    all_in_names = list(in_names) + list(out_names)
    if partition_name is not None:
        all_in_names.append(partition_name)

    def _body(*args):
        operands = list(args)
        if partition_name is not None:
            operands.append(partition_id_tensor())
        outs = _bass_exec_p.bind(
            *operands,
            out_avals=tuple(out_avals),
            in_names=tuple(all_in_names),
            out_names=tuple(out_names),
            lowering_input_output_aliases=(),
            sim_require_finite=True,
            sim_require_nnan=True,
            nc=nc,
        )
        return tuple(outs)

    devices = jax.devices()[:n_cores]
    mesh = Mesh(np.asarray(devices), ("core",))
    donate = tuple(range(n_params, n_params + n_outs))
    sharded = jax.jit(
        shard_map(_body, mesh=mesh,
                  in_specs=(PartitionSpec("core"),) * (n_params + n_outs),
                  out_specs=(PartitionSpec("core"),) * n_outs,
                  check_rep=False),
        donate_argnums=donate, keep_unused=True)
    sharding = NamedSharding(mesh, PartitionSpec("core"))
    per_core = [[np.asarray(m[name]) for name in in_names] for m in in_maps]
    concat_in = [
        np.concatenate([per_core[c][i] for c in range(n_cores)], axis=0)
        for i in range(n_params)]
    concat_zeros = [
        np.zeros((n_cores * z.shape[0], *z.shape[1:]), z.dtype)
        for z in zero_outs]

    def stage_in():
        dev_in = [jax.device_put(a, sharding) for a in concat_in]
        jax.block_until_ready(dev_in)
        return dev_in

    def stage_zeros():
        dev_zeros = [jax.device_put(z, sharding) for z in concat_zeros]
        jax.block_until_ready(dev_zeros)
        return dev_zeros

    return sharded, stage_in, stage_zeros, out_names, out_avals


V2 = True  # use the aggregate-first kernel (build_kernel2)
REPK = 12  # marginal timing reference: repeat=REPK vs repeat=1


def run_timed(inputs, rounds=10, iters_per_round=5):
    """Measure the kernel's on-device execution time via the marginal-wall
    method: build the network once (repeat=1) and twice back-to-back
    (repeat=2) in one NEFF each, time both executables with inputs staged on
    device (donated zero output buffers staged untimed per call), and report
    best2 - best1 per adjacent round (robust to slow drift in the axon
    dispatch-floor, which is ~50-130x the kernel time and excluded by the
    subtraction).  The NTFF profiling path is unavailable under this axon
    client; this is the closest honest proxy for the HW execution span.

    Returns (full output ndarray, marginal seconds, walls1, walls2)."""
    import time
    import jax

    x = np.asarray(inputs["x"], dtype=np.float32)
    N, DIN = x.shape
    F1 = inputs["W1"].shape[1]
    F2 = inputs["W2"].shape[1]
    F3 = inputs["W3"].shape[1]
    edge_index = np.asarray(inputs["edge_index"])
    plan = make_plan(edge_index, N, sort_src=V2)
    build = build_kernel2 if V2 else build_kernel
    mkmaps = make_in_maps2 if V2 else make_in_maps
    nc1 = build(plan, DIN, F1, F2, F3, repeat=1)
    nc2 = build(plan, DIN, F1, F2, F3, repeat=REPK)
    in_maps = mkmaps(plan, inputs, DIN, F1, F2, F3)
    n_cores = plan["n_cores"]

    class R:
        def __init__(self, nc):
            (self.sharded, stage_in, self.stage_zeros, self.out_names,
             self.out_avals) = _sharded_runner(nc, in_maps)
            self.dev_in = stage_in()
            self.last_out = None

    def timed_calls(r, n):
        ws = []
        for _ in range(n):
            dev_zeros = r.stage_zeros()
            t0 = time.perf_counter()
            oa = r.sharded(*r.dev_in, *dev_zeros)
            jax.block_until_ready(oa)
            ws.append(time.perf_counter() - t0)
            r.last_out = oa
        return ws

    r1 = R(nc1)
    r2 = R(nc2)
    timed_calls(r1, 2)  # warm (load executables)
    timed_calls(r2, 2)

    walls1, walls2, marginals = [], [], []
    for _ in range(rounds):
        w1 = timed_calls(r1, iters_per_round)
        w2 = timed_calls(r2, iters_per_round)
        walls1 += w1
        walls2 += w2
        marginals.append((min(w2) - min(w1)) / (REPK - 1))
    marginals.sort()
    marginal = marginals[len(marginals) // 2]
    marginal = max(marginal, 2e-5)

    i_out = r1.out_names.index("out")
    npc = plan["npc"]
    out = np.asarray(r1.last_out[i_out]).reshape(
        n_cores * npc, F3).astype(np.float32)
    out2 = np.asarray(r2.last_out[i_out]).reshape(
        n_cores * npc, F3).astype(np.float32)
    assert np.allclose(out, out2, atol=1e-3, rtol=1e-2), \
        "repeat=2 executable disagrees with repeat=1"
    return out, marginal, walls1, walls2


def kernel(**inputs):
    x = np.asarray(inputs["x"], dtype=np.float32)
    N, DIN = x.shape
    F1 = inputs["W1"].shape[1]
    F2 = inputs["W2"].shape[1]
    F3 = inputs["W3"].shape[1]
    edge_index = np.asarray(inputs["edge_index"])

    key = (N, DIN, F1, F2, F3, hash(edge_index.tobytes()))
    if key not in _CACHE:
        plan = make_plan(edge_index, N, sort_src=V2)
        nc = (build_kernel2 if V2 else build_kernel)(plan, DIN, F1, F2, F3)
        _CACHE[key] = (plan, nc)
    plan, nc = _CACHE[key]

    in_maps = (make_in_maps2 if V2 else make_in_maps)(
        plan, inputs, DIN, F1, F2, F3)
    res = run_bass_kernel_spmd(nc, in_maps, core_ids=list(range(plan["n_cores"])))
    out = np.concatenate([res.results[r]["out"] for r in range(plan["n_cores"])],
                         axis=0)
    return out.astype(np.float32)


if __name__ == "__main__":
    import reference

    inputs = {k: np.asarray(v) for k, v in reference.setup_inputs().items()}
    out = kernel(**inputs)
    exp = np.asarray(reference.reference(**inputs))
    err = np.abs(out - exp).max() / (np.abs(exp).max() + 1e-30)
    print("Relative error:", err)

